# revision 32
# baseline (speedup 1.0000x reference)
"""Multi-head attention forward (B=2, S=2048, H=2048, 16 heads) on 8 TRN2 NeuronCores.

Sharding: tensor-parallel over heads — 2 heads per core. Each core computes
Q/K/V projections for its 2 heads (full batch), attention, and a partial
output projection (its heads' columns of Wo); the host sums the 8 partial
outputs and adds the bias terms.

Device compute is bf16 with fp32 PSUM accumulation. Host pre-transposes
the activation matrix (X.T) and weight slices so the device never has to
transpose fp32 data (fp32 DMA transpose is unsupported).

Layout notes (matmul computes lhsT.T @ rhs, contracting the partition dim):
  - Q.T, K.T are computed as [head_dim, tokens] (d on partitions):
        lhsT = Wq.T tile [hid, d], rhs = X.T tile [hid, tokens]
  - V is computed natural [tokens, d]: lhsT = X.T tile, rhs = Wv.T tile
  - scores transposed S.T[k_tok, q] = (K.T tile).T @ Q.T  (contract d=128)
  - P.T = exp(SCALE * S.T + mask) via one scalar-engine activation
    (mask is per-key = per-partition, so it rides the activation bias;
    with an all-zero mask, two score tiles share one [128,1024] exp)
  - ctx.T[d, q] = V_tile.T @ P.T (contract k_tok), accumulated over k tiles
  - softmax denominators accumulate on DVE (bf16 accumulator tiles
    trailing the exps) with one ones-vector matmul per attention block
  - 1/sums broadcast across partitions via a ones-row matmul (norm tail
    deferred into the next block so no engine ever waits on the chain)
  - out_partial[t, o] = (ctx.T tile).T @ Wo.T tile (contract local head dims)

Schedule (_build_nc2): the PE instruction stream is a single sequence in
which projection and output-projection matmuls are interleaved at key-tile
granularity inside the attention blocks ("filler"), so the tensor engine
never waits on the scalar engine's exps. Emission order: chunks 0-3 (batch
0 projections) -> b0 attention blocks (qi-major, chunk 4-7 filler) -> drain
remaining chunk work interleaved with b0 output projections (psum drains on
the otherwise idle ACT engine) -> b1 attention blocks (outproj filler) ->
tail. PE busy is within ~2% of the bf16 matmul floor for this layout.

bv/bo are folded on the host: rows of normalized P sum to 1, so
ctx = P@(V + bv) = P@V + bv, giving out += bv @ Wo.T + bo after the
cross-core reduction.

Measured (same-session A/B slope benchmarking, bench.py): v2 pair_exp
~462us/iter vs the v1 schedule ~554us/iter on the same device state
(the graded single-shot baseline for v1 was 502897ns).

v3 (2026-08-11): softmax denominators moved from the per-exp DVE
accumulate chain to a deferred DVE adder tree over the block's retained
exp tiles (sums_tree2, emitted at the next block's start). The chain's
per-op waits on fresh ACT exps serialized the DVE queue and blocked the
outproj drains queued behind it; the tree's inputs are all long-ready.
Paired interleaved A/B (14 rounds, K=129 in-NEFF repeats): -17us vs v2
on the same device state. HW microbenchmarks (micro.py): 512-wide bf16
MM 238-250ns (vs 216 ideal; +25ns/MM per extra interleaved PSUM-bank
accumulation group), exp[128,1024] PSUM->SBUF 976ns, DVE add[128,1024]
bf16 678ns (2x mode does not engage). Full-kernel PE busy is ~380us at
these rates, so the schedule is within ~8-14% of this algorithm's
per-instruction PE floor. Also built and HW-rejected: per-kt PE
ones-matmul sums (+90us: ldweights alternation + a PE->DVE->PE psum-bank
cycle), cross-block exp/consume streaming (pt-pool pressure, no win),
4-kt super-step batching and DMA-broadcast norm (no replicated win over
plain sums_tree2).
"""

import os

import numpy as np
import ml_dtypes

P = 128
HIDDEN = 2048
NUM_HEADS = 16
HEAD_DIM = 128
B, S = 2, 2048
T = B * S                     # 4096 tokens
N_CORES = 8
H_LOC = NUM_HEADS // N_CORES  # 2 heads per core
DLOC = H_LOC * HEAD_DIM       # 256
KO = HIDDEN // P              # 16 contraction tiles for the projections
CH = 8                        # token chunks for the projection phase
CHW = T // CH                 # 512 tokens per chunk
NKT = S // P                  # 16 key tiles per batch
NQ = S // 512                 # 4 query tiles (512 wide) per batch
SCALE = float(1.0 / np.sqrt(HEAD_DIM).astype(np.float32))

BF16NP = ml_dtypes.bfloat16

_CACHE = {}


def _split_multi_waits(nc):
    """Split instructions carrying >1 semaphore wait.

    This walrus build rejects any instruction with more than one sync wait
    ("Too many sync wait commands"), but Tile's wait assignment freely
    attaches several. Hoist all but the last wait onto same-engine NOPs
    inserted immediately before the instruction — each engine sequencer
    executes its queue in order, so blocking on a preceding NOP is
    equivalent to blocking on the instruction itself.
    """
    import bass_rust
    import concourse.mybir as mybir

    cnt = 0
    for f in nc.m.functions:
        for bb in f.blocks:
            out = []
            for inst in bb.instructions:
                si = inst.sync_info
                waits = list(si.on_wait) if si and si.on_wait else []
                if len(waits) > 1:
                    for w in waits[:-1]:
                        nop = mybir.InstNoOp(name=f"wsplit_{cnt}", ins=[], outs=[])
                        cnt += 1
                        nop.engine = inst.engine
                        nop.sync_info = bass_rust.SyncInfo(on_wait=[w], on_update=[])
                        out.append(nop)
                    inst.sync_info = bass_rust.SyncInfo(
                        on_wait=[waits[-1]], on_update=list(si.on_update or [])
                    )
                out.append(inst)
            bb.instructions[:] = out
    return cnt


def _build_nc(loop_k=None, **opts):
    """Build the kernel module.

    loop_k: if set, wrap the whole compute body in a For_i running it loop_k
    times — used only for benchmarking (slope timing); the graded kernel
    uses loop_k=None (straight-line body).
    opts: benchmark-only ablation switches (default: all off).
    """
    import concourse.bass as bass
    import concourse.mybir as mybir
    import concourse.tile as tile

    no_sums = opts.get("no_sums", False)
    no_phase3 = opts.get("no_phase3", False)
    no_out_dma = opts.get("no_out_dma", False)
    no_attn = opts.get("no_attn", False)
    xch_bufs = opts.get("xch_bufs", 3)
    pt_bufs = opts.get("pt_bufs", 4)
    norm2 = opts.get("norm2", False)        # deferred norm (early psum drain)
    interleave = opts.get("interleave", False)  # phase 2/3 interleaved per batch
    fuse = opts.get("fuse", False)          # phase 2/3 fused at qi granularity
    vcopy_act = opts.get("vcopy_act", False)  # V psum drain on scalar engine
    norm_gp = opts.get("norm_gp", False)    # norm bounce DMAs on ACT queues
    xt_gp = opts.get("xt_gp", False)        # xt streaming loads on ACT queues
    norm3 = opts.get("norm3", False)        # reciprocal broadcast via PE matmul
    sched2 = opts.get("sched2", False)      # global proj/attn/outproj interleave
    stagger = opts.get("stagger", 0)        # ctx MM issued N steps behind S.T
    act_lite = opts.get("act_lite", False)  # keep ACT for exps only
    early_x = opts.get("early_x", False)    # first x chunk loads before consts
    sums_defer = opts.get("sums_defer", False)  # sums MMs after the kt loop
    sums_tree = opts.get("sums_tree", None)  # "gpsimd"|"vector": adder tree
    sums_acc = opts.get("sums_acc", None)   # "gpsimd"|"vector": inline accum
    ps_bufs = opts.get("ps_bufs", 8)
    out_bf16 = opts.get("out_bf16", False)  # bf16 partial output
    obp_bufs = opts.get("obp_bufs", 3)
    split_in = opts.get("split_in", False)  # split startup DMAs for fast ramp

    fp32 = mybir.dt.float32
    bf16 = mybir.dt.bfloat16

    nc = bass.Bass()

    xt_d = nc.dram_tensor("xt", [HIDDEN, T], bf16, kind="ExternalInput")
    wqt_d = nc.dram_tensor("wqt", [HIDDEN, DLOC], bf16, kind="ExternalInput")
    wkt_d = nc.dram_tensor("wkt", [HIDDEN, DLOC], bf16, kind="ExternalInput")
    wvt_d = nc.dram_tensor("wvt", [HIDDEN, DLOC], bf16, kind="ExternalInput")
    wot_d = nc.dram_tensor("wot", [DLOC, HIDDEN], bf16, kind="ExternalInput")
    bq_d = nc.dram_tensor("bq", [DLOC], fp32, kind="ExternalInput")
    bk_d = nc.dram_tensor("bk", [DLOC], fp32, kind="ExternalInput")
    mask_d = nc.dram_tensor("mask", [B, S], fp32, kind="ExternalInput")
    out_dt = bf16 if out_bf16 else fp32
    out_d = nc.dram_tensor("out", [T, HIDDEN], out_dt, kind="ExternalOutput")

    xt_v = xt_d[:].rearrange("(ko p) t -> p ko t", p=P)
    wqt_v = wqt_d[:].rearrange("(ko p) d -> p ko d", p=P)
    wkt_v = wkt_d[:].rearrange("(ko p) d -> p ko d", p=P)
    wvt_v = wvt_d[:].rearrange("(ko p) d -> p ko d", p=P)
    wot_v = wot_d[:].rearrange("(h p) o -> p h o", p=P)
    bq_v = bq_d[:].rearrange("(h p) -> p h", p=P)
    bk_v = bk_d[:].rearrange("(h p) -> p h", p=P)
    mask_v = mask_d[:].rearrange("b (ko p) -> p b ko", p=P)

    with tile.TileContext(nc) as tc:
        with (
            tc.tile_pool(name="const", bufs=1) as const,
            tc.tile_pool(name="big", bufs=1) as big,
            tc.tile_pool(name="xch", bufs=xch_bufs) as xch,
            tc.tile_pool(name="ptp", bufs=pt_bufs) as ptp,
            tc.tile_pool(name="nrm", bufs=opts.get("nrm_bufs", 2)) as nrm,
            tc.tile_pool(name="ob", bufs=obp_bufs) as obp,
            tc.tile_pool(name="ps", bufs=ps_bufs, space="PSUM") as psp,
            tc.tile_pool(name="dscr", bufs=4, space="DRAM") as dscr,
        ):
            Ident = mybir.ActivationFunctionType.Identity
            Exp = mybir.ActivationFunctionType.Exp

            def ps_tile():
                return psp.tile([P, 512], fp32, tag="ps", name="ps")

            # ---- resident constants -------------------------------------
            wq_sb = const.tile([P, KO, DLOC], bf16)
            wk_sb = const.tile([P, KO, DLOC], bf16)
            wv_sb = const.tile([P, KO, DLOC], bf16)
            wo_sb = const.tile([P, H_LOC, HIDDEN], bf16)
            bq_sb = const.tile([P, H_LOC], fp32)
            bk_sb = const.tile([P, H_LOC], fp32)
            mask_sb = const.tile([P, B, NKT], fp32)
            ones_sb = const.tile([P, 1], bf16)
            ones_row = const.tile([1, P], bf16)   # lhsT for rcp broadcast MM

            xc0 = None
            if split_in:
                for ko in range(0, KO, 4):
                    nc.sync.dma_start(wq_sb[:, ko:ko + 4, :], wqt_v[:, ko:ko + 4, :])
                if early_x:
                    # queue the first activation chunk ahead of the remaining
                    # constants so the first projection matmuls start early
                    xc0 = xch.tile([P, KO, CHW], bf16, tag="xc", name="xc")
                    for ko in range(0, KO, 4):
                        nc.sync.dma_start(
                            xc0[:, ko:ko + 4, :], xt_v[:, ko:ko + 4, 0:CHW]
                        )
                for ko in range(0, KO, 4):
                    nc.sync.dma_start(wk_sb[:, ko:ko + 4, :], wkt_v[:, ko:ko + 4, :])
                for ko in range(0, KO, 4):
                    nc.sync.dma_start(wv_sb[:, ko:ko + 4, :], wvt_v[:, ko:ko + 4, :])
            else:
                nc.sync.dma_start(wq_sb[:], wqt_v)
                nc.sync.dma_start(wk_sb[:], wkt_v)
                nc.sync.dma_start(wv_sb[:], wvt_v)
            nc.sync.dma_start(wo_sb[:], wot_v)
            nc.sync.dma_start(bq_sb[:], bq_v)
            nc.sync.dma_start(bk_sb[:], bk_v)
            nc.sync.dma_start(mask_sb[:], mask_v)
            nc.vector.memset(ones_sb[:], 1.0)
            nc.vector.memset(ones_row[:], 1.0)

            # ---- big activation buffers ---------------------------------
            qt_sb = big.tile([P, H_LOC, T], bf16)   # Q.T  (d on partitions)
            kt_sb = big.tile([P, H_LOC, T], bf16)   # K.T
            v_sb = big.tile([P, T // P, DLOC], bf16)  # V natural (t on partitions)
            ctx_sb = big.tile([P, H_LOC, T], bf16)  # ctx.T

            def emit_body():
                if sched2:
                    # Global interleave: keep ACT-independent matmul work
                    # (projections / output projection) flowing between
                    # attention blocks so exp latency never stalls PE.
                    emit_phase1(range(0, 4))
                    att_b0 = [(0, h, qi) for h in range(H_LOC) for qi in range(NQ)]
                    for i, c in enumerate(range(4, CH)):
                        emit_phase1([c])
                        for blk in att_b0[2 * i:2 * i + 2]:
                            attention(*blk)
                    att_b1 = [(1, h, qi) for h in range(H_LOC) for qi in range(NQ)]
                    for i, blk in enumerate(att_b1):
                        attention(*blk)
                        if not no_phase3:
                            outproj(2 * i, copy_eng=0)
                            outproj(2 * i + 1, copy_eng=1)
                    if not no_phase3:
                        for tt in range(T // P // 2, T // P):
                            outproj(tt, copy_eng=tt % 2)
                    return
                emit_phase1()
                if fuse:
                    # qi-granular fusion: as soon as both heads of a q-tile
                    # are done, run its output projection + store.
                    for b in range(B):
                        for qi in range(NQ):
                            for h in range(H_LOC):
                                attention(b, h, qi)
                            if not no_phase3:
                                for j in range(4):
                                    outproj(b * 16 + qi * 4 + j, copy_eng=j % 2)
                elif interleave:
                    emit_phase2([0])
                    emit_phase3(range(0, T // P // 2))
                    emit_phase2([1])
                    emit_phase3(range(T // P // 2, T // P))
                else:
                    emit_phase2()
                    emit_phase3()

            # ---- phase 1: Q/K/V projections, streamed over token chunks --
            def emit_phase1(cs=tuple(range(CH))):
              for c in cs:
                if c == 0 and xc0 is not None:
                    xc = xc0
                else:
                    xc = xch.tile([P, KO, CHW], bf16, tag="xc", name="xc")
                    xt_eng = nc.scalar if xt_gp else nc.sync
                    if split_in:
                        for ko in range(0, KO, 4):
                            xt_eng.dma_start(
                                xc[:, ko:ko + 4, :],
                                xt_v[:, ko:ko + 4, c * CHW:(c + 1) * CHW],
                            )
                    else:
                        xt_eng.dma_start(xc[:], xt_v[:, :, c * CHW:(c + 1) * CHW])

                for h in range(H_LOC):
                    hd = slice(h * P, (h + 1) * P)
                    psq = ps_tile()
                    for ko in range(KO):
                        nc.tensor.matmul(
                            psq[:], wq_sb[:, ko, hd], xc[:, ko, :],
                            start=(ko == 0), stop=(ko == KO - 1),
                        )
                    if act_lite:
                        nc.vector.tensor_scalar_add(
                            qt_sb[:, h, c * CHW:(c + 1) * CHW], psq[:],
                            bq_sb[:, h:h + 1],
                        )
                    else:
                        nc.scalar.activation(
                            qt_sb[:, h, c * CHW:(c + 1) * CHW], psq[:],
                            Ident, bias=bq_sb[:, h:h + 1],
                        )
                    psk = ps_tile()
                    for ko in range(KO):
                        nc.tensor.matmul(
                            psk[:], wk_sb[:, ko, hd], xc[:, ko, :],
                            start=(ko == 0), stop=(ko == KO - 1),
                        )
                    if act_lite:
                        nc.vector.tensor_scalar_add(
                            kt_sb[:, h, c * CHW:(c + 1) * CHW], psk[:],
                            bk_sb[:, h:h + 1],
                        )
                    else:
                        nc.scalar.activation(
                            kt_sb[:, h, c * CHW:(c + 1) * CHW], psk[:],
                            Ident, bias=bk_sb[:, h:h + 1],
                        )

                for tt in range(CHW // P):
                    psv = ps_tile()
                    for ko in range(KO):
                        nc.tensor.matmul(
                            psv[:, :DLOC], xc[:, ko, tt * P:(tt + 1) * P],
                            wv_sb[:, ko, :],
                            start=(ko == 0), stop=(ko == KO - 1),
                        )
                    if vcopy_act:
                        nc.scalar.copy(v_sb[:, c * (CHW // P) + tt, :], psv[:, :DLOC])
                    else:
                        nc.vector.tensor_copy(
                            v_sb[:, c * (CHW // P) + tt, :], psv[:, :DLOC]
                        )

            # ---- phase 2: attention for one (batch, head, q-tile) --------
            def attention(b, h, qi):
                hd = slice(h * P, (h + 1) * P)
                qs = slice(b * S + qi * 512, b * S + (qi + 1) * 512)
                ps_ctx = ps_tile()
                ps_sum = ps_tile()
                pts = []
                accs = [None, None]
                aeng = None
                if sums_acc is not None:
                    aeng = nc.gpsimd if sums_acc == "gpsimd" else nc.vector
                def emit_st_exp(kt):
                    ks = slice(b * S + kt * P, b * S + (kt + 1) * P)
                    ps_s = ps_tile()
                    nc.tensor.matmul(
                        ps_s[:], kt_sb[:, h, ks], qt_sb[:, h, qs],
                        start=True, stop=True,
                    )
                    pt = ptp.tile([P, 512], bf16, tag="pt", name="pt")
                    nc.scalar.activation(
                        pt[:], ps_s[:], Exp,
                        bias=mask_sb[:, b, kt:kt + 1], scale=SCALE,
                    )
                    pts.append(pt)

                def emit_consume(kt):
                    pt = pts[kt]
                    nc.tensor.matmul(
                        ps_ctx[:], v_sb[:, b * NKT + kt, hd], pt[:],
                        start=(kt == 0), stop=(kt == NKT - 1),
                    )
                    if no_sums:
                        return
                    if sums_acc is not None:
                        # two interleaved accumulators trail the exps
                        i = kt % 2
                        if accs[i] is None:
                            accs[i] = ptp.tile(
                                [P, 512], bf16, tag=f"sacc{i}",
                                name=f"sacc{i}", bufs=2,
                            )
                            aeng.tensor_copy(accs[i][:], pt[:])
                        else:
                            aeng.tensor_add(accs[i][:], accs[i][:], pt[:])
                    elif not sums_defer and sums_tree is None:
                        nc.tensor.matmul(
                            ps_sum[0:1, :], ones_sb[:], pt[:],
                            start=(kt == 0), stop=(kt == NKT - 1),
                        )

                for kt in range(NKT):
                    emit_st_exp(kt)
                    if not no_attn and kt >= stagger:
                        emit_consume(kt - stagger)
                if not no_attn:
                    for kt in range(NKT - stagger, NKT):
                        emit_consume(kt)
                if no_attn:
                    return
                if sums_acc is not None and not no_sums:
                    aeng.tensor_add(accs[0][:], accs[0][:], accs[1][:])
                    nc.tensor.matmul(
                        ps_sum[0:1, :], ones_sb[:], accs[0][:],
                        start=True, stop=True,
                    )
                if sums_defer and not no_sums:
                    for kt in range(NKT):
                        nc.tensor.matmul(
                            ps_sum[0:1, :], ones_sb[:], pts[kt][:],
                            start=(kt == 0), stop=(kt == NKT - 1),
                        )
                if sums_tree is not None and not no_sums:
                    # Pairwise-add the 16 exp tiles on a non-PE engine, then a
                    # single ones-matmul does the partition reduction.
                    teng = nc.gpsimd if sums_tree == "gpsimd" else nc.vector
                    lvl = list(pts)
                    li = 0
                    while len(lvl) > 1:
                        nxt = []
                        for i in range(0, len(lvl), 2):
                            t = ptp.tile(
                                [P, 512], bf16, tag=f"tl{li}", name=f"tl{li}",
                                bufs=(10 if li == 0 else 5),
                            )
                            teng.tensor_add(t[:], lvl[i][:], lvl[i + 1][:])
                            nxt.append(t)
                        lvl = nxt
                        li += 1
                    nc.tensor.matmul(
                        ps_sum[0:1, :], ones_sb[:], lvl[0][:],
                        start=True, stop=True,
                    )
                if no_sums:
                    nc.vector.tensor_copy(ctx_sb[:, h, qs], ps_ctx[:])
                    return
                rcp = nrm.tile([1, 512], fp32, tag="rcp", name="rcp")
                nc.vector.reciprocal(rcp[:], ps_sum[0:1, :])
                if norm3:
                    # Broadcast 1/sums across partitions with one K=1 matmul
                    # (ones_row.T @ rcp) — no DMA round trip on the critical
                    # path to ctx_sb.
                    rcpb = nrm.tile([1, 512], bf16, tag="rcpb", name="rcpb")
                    nc.vector.tensor_copy(rcpb[:], rcp[:])
                    ps_rbc = ps_tile()
                    nc.tensor.matmul(
                        ps_rbc[:], ones_row[:], rcpb[:], start=True, stop=True,
                    )
                    ctxu = nrm.tile([P, 512], fp32, tag="ctxu", name="ctxu")
                    nc.vector.tensor_copy(ctxu[:], ps_ctx[:])
                    nc.vector.tensor_mul(ctx_sb[:, h, qs], ctxu[:], ps_rbc[:])
                    return
                rbc = nrm.tile([P, 512], fp32, tag="rbc", name="rbc")
                rdr = dscr.tile([1, 512], fp32, tag="rdr", name="rdr")
                dma_eng = nc.scalar if norm_gp else nc.sync
                if norm2:
                    # Drain the ctx psum to SBUF right away (frees the
                    # bank); the reciprocal broadcast (DRAM bounce)
                    # happens off the critical path.
                    ctxu = nrm.tile([P, 512], fp32, tag="ctxu", name="ctxu")
                    nc.vector.tensor_copy(ctxu[:], ps_ctx[:])
                    dma_eng.dma_start(rdr[:], rcp[:])
                    dma_eng.dma_start(rbc[:], rdr[:].to_broadcast((P, 512)))
                    nc.vector.tensor_mul(ctx_sb[:, h, qs], ctxu[:], rbc[:])
                else:
                    dma_eng.dma_start(rdr[:], rcp[:])
                    dma_eng.dma_start(rbc[:], rdr[:].to_broadcast((P, 512)))
                    nc.vector.tensor_mul(ctx_sb[:, h, qs], ps_ctx[:], rbc[:])

            def emit_phase2(bs=tuple(range(B))):
                for b in bs:
                    for h in range(H_LOC):
                        for qi in range(NQ):
                            attention(b, h, qi)

            # ---- phase 3: partial output projection ----------------------
            def outproj(tt, copy_eng=0):
                ts_ = slice(tt * P, (tt + 1) * P)
                for oi in range(HIDDEN // 512):
                    os_ = slice(oi * 512, (oi + 1) * 512)
                    ps_o = ps_tile()
                    for h in range(H_LOC):
                        nc.tensor.matmul(
                            ps_o[:], ctx_sb[:, h, ts_], wo_sb[:, h, os_],
                            start=(h == 0), stop=(h == H_LOC - 1),
                        )
                    ob = obp.tile([P, 512], out_dt, tag="ob", name="ob")
                    if not act_lite and (copy_eng + oi) % 2:
                        nc.scalar.copy(ob[:], ps_o[:])
                    else:
                        nc.vector.tensor_copy(ob[:], ps_o[:])
                    if not no_out_dma:
                        nc.sync.dma_start(out_d[ts_, os_], ob[:])

            def emit_phase3(tts=tuple(range(T // P))):
                if no_phase3:
                    return
                for tt in tts:
                    outproj(tt)

            if loop_k is None:
                emit_body()
            else:
                with tc.For_i(0, loop_k, 1):
                    emit_body()

    _split_multi_waits(nc)
    return nc


def _build_nc2(loop_k=None, **opts):
    """v2 schedule.

    Differences from v1:
      - softmax denominators accumulate on DVE (two interleaved bf16
        accumulators trailing the exps) with a single ones-matmul per
        block, removing ~51us of PE ones-matmul work;
      - projection (chunks 4-7) and output-projection matmuls are fed
        into the attention blocks as per-kt filler so PE keeps streaming
        while ACT produces exps;
      - b0/b1 attention blocks run qi-major and output projections are
        appended as soon as both heads of a q-tile are normalized, which
        spreads the store traffic and shrinks the tail.
    """
    import concourse.bass as bass
    import concourse.mybir as mybir
    import concourse.tile as tile

    stagger = opts.get("stagger", 6)
    fill_b0 = opts.get("fill_b0", 1)
    fill_b1 = opts.get("fill_b1", 2)
    xch_bufs = opts.get("xch_bufs", 3)
    pt_bufs = opts.get("pt_bufs", 14)
    ps_bufs = opts.get("ps_bufs", 8)
    obp_bufs = opts.get("obp_bufs", 16)
    acc_bufs = opts.get("acc_bufs", 2)
    s1_at = opts.get("s1_at", 6)
    s2_at = opts.get("s2_at", 10)
    norm_dma = opts.get("norm_dma", False)
    norm_pool = opts.get("norm_pool", False)  # denominator reduce on gpsimd
    # pair_exp: fuse each pair of score tiles into one [128,1024] exp (double
    # psum bank read). Only valid when the attention mask is all-zero (the
    # per-key bias column differs between the two tiles otherwise); kernel()
    # selects it at build time after inspecting the mask.
    pair_exp = opts.get("pair_exp", False)
    # defer_q: split chunks 4-7 into K/V and Q passes, keeping Q of chunks
    # 6-7 as cheap filler for the early b1 attention blocks
    defer_q = opts.get("defer_q", True)
    # drain_to: which chunk's Q pass the pre-b1 drain phase runs through
    # (later Q passes become b1-block filler, pulled under emission guards)
    drain_to = opts.get("drain_to", 5)
    tail_split = opts.get("tail_split", False)  # half-tile tail drains/stores
    drain_ratio = opts.get("drain_ratio", 3)
    alt_mod = opts.get("alt_mod", 4)  # 1 of alt_mod b1 drains goes to ACT
    # NOTE: gpsimd cannot read PSUM in this toolchain (compile fails), so
    # outproj drains must stay on ACT/DVE.
    b1_pool = opts.get("b1_pool", False)
    out_bf16 = opts.get("out_bf16", True)
    # bench-only ablations (never set by kernel(); numerics are wrong):
    abl_no_store = opts.get("abl_no_store", False)   # skip out DMA stores
    abl_no_xt = opts.get("abl_no_xt", False)         # chunks>0 reuse xc0
    abl_half_exp = opts.get("abl_half_exp", False)   # alias every other exp
    abl_no_outproj = opts.get("abl_no_outproj", False)
    abl_proj_only = opts.get("abl_proj_only", False)  # phase 1 only
    abl_attn_only = opts.get("abl_attn_only", False)  # phase 2 only (memset QKV)
    abl_no_acc = opts.get("abl_no_acc", False)   # no DVE sums, raw ctx copy
    abl_raw_ctx = opts.get("abl_raw_ctx", False)  # sums kept, no norm mul
    acc_pool = opts.get("acc_pool", False)  # acc chain on gpsimd (correct)
    # sums_pe: softmax denominators via per-kt ones-matmuls accumulating in
    # a dedicated PSUM bank (PE-local chain after each ctx matmul), instead
    # of the DVE accumulate chain. Removes the ACT->DVE sem coupling that
    # stalls the DVE queue (and everything behind it) on HW.
    sums_pe = opts.get("sums_pe", False)
    # sums_tree2: keep all 8 exp pair-tiles of a block alive and reduce them
    # with a deferred DVE adder tree at the START of the next block (s0),
    # followed by one ones-matmul (s1). Same DVE op count as the chain but
    # all inputs are long-ready: no per-op ACT wait, no queue head-of-line.
    sums_tree2 = opts.get("sums_tree2", False)
    s0_at = opts.get("s0_at", 1)
    # stream_attn: software-pipeline attention ACROSS blocks — the score/exp
    # stream runs a GLOBAL `stagger` steps ahead of the ctx-consume stream,
    # so block seams have no consume-tail during which ACT starves.
    # Requires pair_exp + sums_tree2, and s0_at >= stagger (a block's
    # finalize only exists once its last consume has been emitted).
    stream_attn = opts.get("stream_attn", False)
    # attn_batch4: emit attention in 4-kt super-steps — 4 score MMs, then 4
    # consecutive ctx MMs, then a filler burst — so PE runs long same-bank
    # stretches. HW charges ~+25ns/MM per additional interleaved PSUM bank
    # (mm_alt2/mm_alt3 microbench), so fine-grained interleave is costly.
    # Requires pair_exp; stagger must be a multiple of 4.
    attn_batch4 = opts.get("attn_batch4", False)

    fp32 = mybir.dt.float32
    bf16 = mybir.dt.bfloat16

    nc = bass.Bass()

    xt_d = nc.dram_tensor("xt", [HIDDEN, T], bf16, kind="ExternalInput")
    wqt_d = nc.dram_tensor("wqt", [HIDDEN, DLOC], bf16, kind="ExternalInput")
    wkt_d = nc.dram_tensor("wkt", [HIDDEN, DLOC], bf16, kind="ExternalInput")
    wvt_d = nc.dram_tensor("wvt", [HIDDEN, DLOC], bf16, kind="ExternalInput")
    wot_d = nc.dram_tensor("wot", [DLOC, HIDDEN], bf16, kind="ExternalInput")
    bq_d = nc.dram_tensor("bq", [DLOC], fp32, kind="ExternalInput")
    bk_d = nc.dram_tensor("bk", [DLOC], fp32, kind="ExternalInput")
    mask_d = nc.dram_tensor("mask", [B, S], fp32, kind="ExternalInput")
    out_dt = bf16 if out_bf16 else fp32
    out_d = nc.dram_tensor("out", [T, HIDDEN], out_dt, kind="ExternalOutput")

    xt_v = xt_d[:].rearrange("(ko p) t -> p ko t", p=P)
    wqt_v = wqt_d[:].rearrange("(ko p) d -> p ko d", p=P)
    wkt_v = wkt_d[:].rearrange("(ko p) d -> p ko d", p=P)
    wvt_v = wvt_d[:].rearrange("(ko p) d -> p ko d", p=P)
    wot_v = wot_d[:].rearrange("(h p) o -> p h o", p=P)
    bq_v = bq_d[:].rearrange("(h p) -> p h", p=P)
    bk_v = bk_d[:].rearrange("(h p) -> p h", p=P)
    mask_v = mask_d[:].rearrange("b (ko p) -> p b ko", p=P)

    with tile.TileContext(nc) as tc:
        with (
            tc.tile_pool(name="const", bufs=1) as const,
            tc.tile_pool(name="big", bufs=1) as big,
            tc.tile_pool(name="xch", bufs=xch_bufs) as xch,
            tc.tile_pool(name="ptp", bufs=pt_bufs) as ptp,
            tc.tile_pool(name="accp", bufs=acc_bufs) as accp,
            tc.tile_pool(name="nrm", bufs=2) as nrm,
            tc.tile_pool(name="ob", bufs=obp_bufs) as obp,
            tc.tile_pool(name="ps", bufs=ps_bufs, space="PSUM") as psp,
            tc.tile_pool(name="pss", bufs=opts.get("pss_bufs", 2),
                         space="PSUM") as pssp,
            tc.tile_pool(name="dscr", bufs=4, space="DRAM") as dscr,
        ):
            Ident = mybir.ActivationFunctionType.Identity
            Exp = mybir.ActivationFunctionType.Exp

            def ps_tile():
                return psp.tile([P, 512], fp32, tag="ps", name="ps")

            # ---- resident constants -------------------------------------
            wq_sb = const.tile([P, KO, DLOC], bf16)
            wk_sb = const.tile([P, KO, DLOC], bf16)
            wv_sb = const.tile([P, KO, DLOC], bf16)
            wo_sb = const.tile([P, H_LOC, HIDDEN], bf16)
            bq_sb = const.tile([P, H_LOC], fp32)
            bk_sb = const.tile([P, H_LOC], fp32)
            mask_sb = const.tile([P, B, NKT], fp32)
            ones_sb = const.tile([P, 1], bf16)
            ones_row = const.tile([1, P], bf16)  # lhsT for rcp broadcast MM

            # Startup: xc0 streams on the ACT-triggered queue while weights
            # stream on the SP queue, in 2-ko pieces interleaved so the first
            # Q matmuls can begin after ~one piece of each.
            xc0 = xch.tile([P, KO, CHW], bf16, tag="xc", name="xc")
            for ko in range(0, KO, 2):
                nc.sync.dma_start(
                    wq_sb[:, ko:ko + 2, :], wqt_v[:, ko:ko + 2, :]
                )
                nc.scalar.dma_start(
                    xc0[:, ko:ko + 2, :], xt_v[:, ko:ko + 2, 0:CHW]
                )
            for ko in range(0, KO, 8):
                nc.sync.dma_start(wk_sb[:, ko:ko + 8, :], wkt_v[:, ko:ko + 8, :])
            for ko in range(0, KO, 8):
                nc.sync.dma_start(wv_sb[:, ko:ko + 8, :], wvt_v[:, ko:ko + 8, :])
            nc.sync.dma_start(bq_sb[:], bq_v)
            nc.sync.dma_start(bk_sb[:], bk_v)
            nc.sync.dma_start(mask_sb[:], mask_v)
            # wo is first needed ~90us in (output projections); its load is
            # deferred into emit_body so it doesn't crowd startup bandwidth
            nc.vector.memset(ones_sb[:], 1.0)
            nc.vector.memset(ones_row[:], 1.0)

            # ---- big activation buffers ---------------------------------
            qt_sb = big.tile([P, H_LOC, T], bf16)     # Q.T (d on partitions)
            kt_sb = big.tile([P, H_LOC, T], bf16)     # K.T
            v_sb = big.tile([P, T // P, DLOC], bf16)  # V natural
            ctx_sb = big.tile([P, H_LOC, T], bf16)    # normalized ctx.T

            # ---- filler queues ------------------------------------------
            # Each queue entry emits exactly one PE matmul (plus trailing
            # drains / DMAs that complete its accumulation group). Index
            # pointers (not pops) so emission guards can reference positions.
            chunkq = []
            cqi = [0]
            outq = []
            oqi = [0]
            out_reserve = opts.get("out_reserve", 2)

            def pull_filler(n):
                # keep a few freshest outproj steps unpulled so PE never
                # reaches an outproj matmul before its ctx norm lands
                for _ in range(n):
                    if cqi[0] < len(chunkq):
                        chunkq[cqi[0]]()
                        cqi[0] += 1
                    elif len(outq) - oqi[0] > out_reserve:
                        outq[oqi[0]]()
                        oqi[0] += 1
                    else:
                        break

            def drain_cq_to(pos):
                while cqi[0] < pos:
                    chunkq[cqi[0]]()
                    cqi[0] += 1

            def drain_oq():
                while oqi[0] < len(outq):
                    outq[oqi[0]]()
                    oqi[0] += 1

            # ---- phase 1: Q/K/V projections for one 512-token chunk -----
            # Chunks 0-3 emit Q+K+V together ("all"); chunks 4-7 are split
            # into a K+V pass and a separate Q pass (own xc load) so the Q
            # work for chunks 6-7 can defer into the early b1 attention
            # blocks as cheap filler (almost no drain traffic).
            xc_tiles = {("a", 0): xc0}
            _next_key = {}

            def load_chunk(key):
                kind, c = key
                if key in xc_tiles or c >= CH:
                    return
                if abl_no_xt:
                    xc_tiles[key] = xc0
                    return
                cs = slice(c * CHW, (c + 1) * CHW)
                xc = xch.tile([P, KO, CHW], bf16, tag="xc", name="xc")
                nc.sync.dma_start(xc[:, :8, :], xt_v[:, :8, cs])
                nc.sync.dma_start(xc[:, 8:, :], xt_v[:, 8:, cs])
                xc_tiles[key] = xc

            def chunk_steps(c, mode="all"):
                st = {}
                cs = slice(c * CHW, (c + 1) * CHW)
                drain_act = c < 4  # ACT is idle before attention starts
                steps = []
                key = ("a" if mode == "all" else mode, c)

                def first_pre():
                    load_chunk(key)
                    st["xc"] = xc_tiles[key]
                    nxt = _next_key.get(key)
                    if nxt is not None:
                        load_chunk(nxt)  # prefetch one pass ahead

                def qk_group(w_sb, b_sb, dst, h, pre=None):
                    g = {}
                    hd = slice(h * P, (h + 1) * P)

                    def mk(ko):
                        def run():
                            if ko == 0:
                                if pre is not None:
                                    pre()
                                g["ps"] = ps_tile()
                            nc.tensor.matmul(
                                g["ps"][:], w_sb[:, ko, hd], st["xc"][:, ko, :],
                                start=(ko == 0), stop=(ko == KO - 1),
                            )
                            if ko == KO - 1:
                                if drain_act:
                                    nc.scalar.activation(
                                        dst[:, h, cs], g["ps"][:], Ident,
                                        bias=b_sb[:, h:h + 1],
                                    )
                                else:
                                    nc.vector.tensor_scalar_add(
                                        dst[:, h, cs], g["ps"][:], b_sb[:, h:h + 1]
                                    )
                        return run

                    return [mk(ko) for ko in range(KO)]

                def v_group(tt):
                    g = {}

                    def mk(ko):
                        def run():
                            if ko == 0:
                                g["ps"] = ps_tile()
                            nc.tensor.matmul(
                                g["ps"][:, :DLOC],
                                st["xc"][:, ko, tt * P:(tt + 1) * P],
                                wv_sb[:, ko, :],
                                start=(ko == 0), stop=(ko == KO - 1),
                            )
                            if ko == KO - 1:
                                nc.vector.tensor_copy(
                                    v_sb[:, c * (CHW // P) + tt, :],
                                    g["ps"][:, :DLOC],
                                )
                        return run

                    return [mk(ko) for ko in range(KO)]

                if mode == "all" and c == 0:
                    # Interleave the two heads at ko granularity so PE
                    # consumes each freshly-DMA'd 2-ko piece with 4 matmuls
                    # instead of 2 — keeps pace with the startup stream.
                    q0 = qk_group(wq_sb, bq_sb, qt_sb, 0, pre=first_pre)
                    q1 = qk_group(wq_sb, bq_sb, qt_sb, 1)
                    for a, bstep in zip(q0, q1):
                        steps += [a, bstep]
                    k0 = qk_group(wk_sb, bk_sb, kt_sb, 0)
                    k1 = qk_group(wk_sb, bk_sb, kt_sb, 1)
                    for a, bstep in zip(k0, k1):
                        steps += [a, bstep]
                elif mode == "all":
                    steps += qk_group(wq_sb, bq_sb, qt_sb, 0, pre=first_pre)
                    steps += qk_group(wq_sb, bq_sb, qt_sb, 1)
                    steps += qk_group(wk_sb, bk_sb, kt_sb, 0)
                    steps += qk_group(wk_sb, bk_sb, kt_sb, 1)
                elif mode == "kv":
                    steps += qk_group(wk_sb, bk_sb, kt_sb, 0, pre=first_pre)
                    steps += qk_group(wk_sb, bk_sb, kt_sb, 1)
                elif mode == "q":
                    steps += qk_group(wq_sb, bq_sb, qt_sb, 0, pre=first_pre)
                    steps += qk_group(wq_sb, bq_sb, qt_sb, 1)
                    return steps
                for tt in range(CHW // P):
                    steps += v_group(tt)
                return steps

            # ---- output projection for one 128-token tile ---------------
            # dr: which engine drains the psum -> bf16 output buffer.
            #   "act": always ACT (used for b0 tiles drained while ACT is
            #          otherwise idle between the attention phases)
            #   "alt": mostly DVE with every 4th on ACT (b1 tiles drained
            #          while ACT is busy with exps)
            def outproj_steps(tt, dr):
                ts_ = slice(tt * P, (tt + 1) * P)
                steps = []
                for oi in range(HIDDEN // 512):
                    g = {}
                    os_ = slice(oi * 512, (oi + 1) * 512)
                    if dr == "act":
                        cp_eng = "act"
                    elif dr == "alt":
                        cp_eng = (
                            "pool" if b1_pool
                            else ("act" if oi % alt_mod == alt_mod - 1 else "dve")
                        )
                    else:  # tail
                        cp_eng = "act" if oi % 2 else "dve"
                    st_eng = nc.scalar if (dr == "tail" and oi % 2 == 0) else nc.sync

                    def mk(h, oi=oi, os_=os_, g=g, cp_eng=cp_eng, st_eng=st_eng,
                           dr=dr):
                        def run():
                            if h == 0:
                                g["ps"] = ps_tile()
                            nc.tensor.matmul(
                                g["ps"][:], ctx_sb[:, h, ts_], wo_sb[:, h, os_],
                                start=(h == 0), stop=(h == H_LOC - 1),
                            )
                            if h == H_LOC - 1:
                                ob = obp.tile(
                                    [P, 512], out_dt, tag="ob", name="ob"
                                )
                                if dr == "tail" and tail_split:
                                    # half-width drains on both engines and
                                    # stores on both queues: the last stores
                                    # start ~a half-copy earlier
                                    ha = slice(os_.start, os_.start + 256)
                                    hb = slice(os_.start + 256, os_.stop)
                                    nc.vector.tensor_copy(
                                        ob[:, 0:256], g["ps"][:, 0:256]
                                    )
                                    if not abl_no_store:
                                        nc.sync.dma_start(
                                            out_d[ts_, ha], ob[:, 0:256]
                                        )
                                    nc.scalar.copy(
                                        ob[:, 256:512], g["ps"][:, 256:512]
                                    )
                                    if not abl_no_store:
                                        nc.scalar.dma_start(
                                            out_d[ts_, hb], ob[:, 256:512]
                                        )
                                    return
                                if cp_eng == "act":
                                    nc.scalar.copy(ob[:], g["ps"][:])
                                elif cp_eng == "pool":
                                    nc.gpsimd.tensor_copy(ob[:], g["ps"][:])
                                else:
                                    nc.vector.tensor_copy(ob[:], g["ps"][:])
                                if not abl_no_store:
                                    st_eng.dma_start(out_d[ts_, os_], ob[:])
                        return run

                    for h in range(H_LOC):
                        steps.append(mk(h))
                return steps

            # ---- attention for one (batch, head, q-tile) ----------------
            # The normalization tail of block k is deferred into block k+1
            # (s1 at kt==2, s2 at kt==6) so the DVE reciprocal chain and the
            # PE broadcast matmul never make either engine wait: by the time
            # each step runs, its producer on the other engine is long done.
            def attention2(b, h, qi, fill, prev, stg=None):
                stg = stagger if stg is None else stg
                hd = slice(h * P, (h + 1) * P)
                qs = slice(b * S + qi * 512, b * S + (qi + 1) * 512)
                st = {}
                pts = [None] * NKT
                acc = [None, None]

                AW = 1024 if pair_exp else 512

                def st_exp(kt):
                    ks = slice(b * S + kt * P, b * S + (kt + 1) * P)
                    if pair_exp:
                        if kt % 2 == 1:
                            return  # emitted with the even half
                        ks2 = slice(b * S + (kt + 1) * P, b * S + (kt + 2) * P)
                        ps_d = psp.tile(
                            [P, 1024], fp32, tag="psd", name="psd", bufs=2
                        )
                        nc.tensor.matmul(
                            ps_d[:, 0:512], kt_sb[:, h, ks], qt_sb[:, h, qs],
                            start=True, stop=True,
                        )
                        nc.tensor.matmul(
                            ps_d[:, 512:1024], kt_sb[:, h, ks2], qt_sb[:, h, qs],
                            start=True, stop=True,
                        )
                        if abl_half_exp and kt % 4 == 2:
                            # timing ablation: PE work identical, exp skipped
                            pts[kt] = pts[kt - 2]
                            pts[kt + 1] = pts[kt - 1]
                            return
                        pt = ptp.tile([P, 1024], bf16, tag="pt", name="pt")
                        nc.scalar.activation(pt[:], ps_d[:], Exp, scale=SCALE)
                        pts[kt] = (pt, 0)
                        pts[kt + 1] = (pt, 512)
                        return
                    ps_s = ps_tile()
                    nc.tensor.matmul(
                        ps_s[:], kt_sb[:, h, ks], qt_sb[:, h, qs],
                        start=True, stop=True,
                    )
                    pt = ptp.tile([P, 512], bf16, tag="pt", name="pt")
                    nc.scalar.activation(
                        pt[:], ps_s[:], Exp,
                        bias=mask_sb[:, b, kt:kt + 1], scale=SCALE,
                    )
                    pts[kt] = (pt, 0)

                def consume(kt):
                    if kt == 0:
                        st["ctx"] = ps_tile()
                        if sums_pe:
                            st["sum"] = pssp.tile(
                                [P, 512], fp32, tag="pssum", name="pssum"
                            )
                    pt, off = pts[kt]
                    nc.tensor.matmul(
                        st["ctx"][:], v_sb[:, b * NKT + kt, hd],
                        pt[:, off:off + 512],
                        start=(kt == 0), stop=(kt == NKT - 1),
                    )
                    if sums_pe:
                        nc.tensor.matmul(
                            st["sum"][0:1, :], ones_sb[:], pt[:, off:off + 512],
                            start=(kt == 0), stop=(kt == NKT - 1),
                        )
                        return
                    if abl_no_acc or sums_tree2:
                        return
                    if pair_exp and kt % 2 == 0:
                        return  # accumulate the pair once, at the odd half
                    i = (kt // (2 if pair_exp else 1)) % 2
                    aeng = nc.gpsimd if acc_pool else nc.vector
                    if acc[i] is None:
                        acc[i] = accp.tile(
                            [P, AW], bf16, tag=f"acc{i}", name=f"acc{i}"
                        )
                        aeng.tensor_copy(acc[i][:], pt[:])
                    else:
                        aeng.tensor_add(acc[i][:], acc[i][:], pt[:])

                if attn_batch4:
                    assert pair_exp and stg % 4 == 0
                    for g in range(0, NKT, 4):
                        st_exp(g)
                        st_exp(g + 2)
                        pull_filler(fill * 4)
                        for sn, sat in (("s0", s0_at), ("s1", s1_at),
                                        ("s2", s2_at)):
                            if g <= sat < g + 4 and prev.get(sn):
                                prev[sn]()
                        if g >= stg:
                            for kt in range(g - stg, g - stg + 4):
                                consume(kt)
                    for kt in range(NKT - stg, NKT):
                        consume(kt)
                else:
                    for kt in range(NKT):
                        st_exp(kt)
                        pull_filler(fill)
                        if kt == s0_at and prev.get("s0"):
                            prev["s0"]()
                        if kt == s1_at and prev.get("s1"):
                            prev["s1"]()
                        if kt == s2_at and prev.get("s2"):
                            prev["s2"]()
                        if kt >= stg:
                            consume(kt - stg)
                    for kt in range(NKT - stg, NKT):
                        consume(kt)

                if abl_no_acc:
                    def s2_raw():
                        nc.vector.tensor_copy(ctx_sb[:, h, qs], st["ctx"][:])
                        if h == H_LOC - 1 and not abl_no_outproj:
                            dr = "act" if b == 0 else (
                                "tail" if qi == NQ - 1 else "alt")
                            for tt in range(b * 16 + qi * 4,
                                            b * 16 + qi * 4 + 4):
                                outq.extend(outproj_steps(tt, dr))
                    return {"s1": None, "s2": s2_raw}
                if sums_tree2:
                    ctxu = nrm.tile([P, 512], fp32, tag="ctxu", name="ctxu")
                    nc.vector.tensor_copy(ctxu[:], st["ctx"][:])
                    cur = {}
                    pair_tiles = [pts[2 * i][0] for i in range(NKT // 2)]

                    def s0():
                        lvl = pair_tiles
                        li = 0
                        while len(lvl) > 1:
                            nxt = []
                            for i in range(0, len(lvl), 2):
                                t = accp.tile(
                                    [P, AW], bf16, tag=f"tl{li}",
                                    name=f"tl{li}",
                                    bufs=(opts.get("tl0_bufs", 4)
                                          if li == 0 else 3),
                                )
                                nc.vector.tensor_add(t[:], lvl[i][:], lvl[i + 1][:])
                                nxt.append(t)
                            lvl = nxt
                            li += 1
                        accf2 = accp.tile([P, 512], bf16, tag="accs",
                                          name="accs", bufs=2)
                        nc.vector.tensor_add(
                            accf2[:], lvl[0][:, 0:512], lvl[0][:, 512:1024]
                        )
                        cur["accf"] = accf2

                    def s1t():
                        ps_sum = ps_tile()
                        nc.tensor.matmul(
                            ps_sum[0:1, :], ones_sb[:], cur["accf"][:],
                            start=True, stop=True,
                        )
                        rcp = nrm.tile([1, 512], fp32, tag="rcp", name="rcp")
                        nc.vector.reciprocal(rcp[:], ps_sum[0:1, :])
                        if norm_dma:
                            rdr = dscr.tile([1, 512], fp32, tag="rdr",
                                            name="rdr")
                            rbc = nrm.tile([P, 512], fp32, tag="rbc",
                                           name="rbc")
                            nc.sync.dma_start(rdr[:], rcp[:])
                            nc.sync.dma_start(
                                rbc[:], rdr[:].to_broadcast((P, 512))
                            )
                            cur["rbc"] = rbc
                            return
                        rcpb = nrm.tile([1, 512], bf16, tag="rcpb", name="rcpb")
                        nc.vector.tensor_copy(rcpb[:], rcp[:])
                        cur["rcpb"] = rcpb

                    def s2t():
                        if norm_dma:
                            nc.vector.tensor_mul(
                                ctx_sb[:, h, qs], ctxu[:], cur["rbc"][:]
                            )
                        else:
                            ps_rbc = ps_tile()
                            nc.tensor.matmul(
                                ps_rbc[:], ones_row[:], cur["rcpb"][:],
                                start=True, stop=True,
                            )
                            nc.vector.tensor_mul(
                                ctx_sb[:, h, qs], ctxu[:], ps_rbc[:]
                            )
                        if h == H_LOC - 1 and not abl_no_outproj:
                            if b == 0:
                                dr = "act"
                            elif qi == NQ - 1:
                                dr = "tail"
                            else:
                                dr = "alt"
                            for tt in range(b * 16 + qi * 4,
                                            b * 16 + qi * 4 + 4):
                                outq.extend(outproj_steps(tt, dr))

                    return {"s0": s0, "s1": s1t, "s2": s2t}
                if sums_pe:
                    accf = None
                else:
                    aeng2 = nc.gpsimd if acc_pool else nc.vector
                    aeng2.tensor_add(acc[0][:], acc[0][:], acc[1][:])
                    if pair_exp:
                        accs = accp.tile([P, 512], bf16, tag="accs", name="accs")
                        nc.vector.tensor_add(
                            accs[:], acc[0][:, 0:512], acc[0][:, 512:1024]
                        )
                        accf = accs
                    else:
                        accf = acc[0]
                ctxu = nrm.tile([P, 512], fp32, tag="ctxu", name="ctxu")
                nc.vector.tensor_copy(ctxu[:], st["ctx"][:])
                cur = {}

                def s1():
                    if sums_pe:
                        rcp = nrm.tile([1, 512], fp32, tag="rcp", name="rcp")
                        nc.vector.reciprocal(rcp[:], st["sum"][0:1, :])
                        rcpb = nrm.tile([1, 512], bf16, tag="rcpb", name="rcpb")
                        nc.vector.tensor_copy(rcpb[:], rcp[:])
                        cur["rcpb"] = rcpb
                        return
                    if norm_pool:
                        import concourse.bass_isa as bass_isa
                        rs = nrm.tile([P, 512], fp32, tag="rs", name="rs")
                        nc.gpsimd.partition_all_reduce(
                            rs[:], accf[:], P, bass_isa.ReduceOp.add
                        )
                        rcp = nrm.tile([1, 512], fp32, tag="rcp", name="rcp")
                        nc.vector.reciprocal(rcp[:], rs[0:1, :])
                        rb = nrm.tile([P, 512], fp32, tag="rb2", name="rb2")
                        nc.gpsimd.partition_broadcast(rb[:], rcp[:], P)
                        cur["rb"] = rb
                        return
                    ps_sum = ps_tile()
                    nc.tensor.matmul(
                        ps_sum[0:1, :], ones_sb[:], accf[:],
                        start=True, stop=True,
                    )
                    rcp = nrm.tile([1, 512], fp32, tag="rcp", name="rcp")
                    nc.vector.reciprocal(rcp[:], ps_sum[0:1, :])
                    if norm_dma:
                        # bounce through DRAM to broadcast 1/sums across
                        # partitions; latency hidden by the s1->s2 deferral
                        rdr = dscr.tile([1, 512], fp32, tag="rdr", name="rdr")
                        rbc = nrm.tile([P, 512], fp32, tag="rbc", name="rbc")
                        nc.sync.dma_start(rdr[:], rcp[:])
                        nc.sync.dma_start(rbc[:], rdr[:].to_broadcast((P, 512)))
                        cur["rbc"] = rbc
                    else:
                        rcpb = nrm.tile([1, 512], bf16, tag="rcpb", name="rcpb")
                        nc.vector.tensor_copy(rcpb[:], rcp[:])
                        cur["rcpb"] = rcpb

                def s2():
                    if abl_raw_ctx:
                        pass  # ctxu already holds raw ctx; no norm multiply
                    elif norm_pool:
                        nc.vector.tensor_mul(
                            ctx_sb[:, h, qs], ctxu[:], cur["rb"][:]
                        )
                    elif norm_dma:
                        nc.vector.tensor_mul(
                            ctx_sb[:, h, qs], ctxu[:], cur["rbc"][:]
                        )
                    else:
                        ps_rbc = ps_tile()
                        nc.tensor.matmul(
                            ps_rbc[:], ones_row[:], cur["rcpb"][:],
                            start=True, stop=True,
                        )
                        nc.vector.tensor_mul(ctx_sb[:, h, qs], ctxu[:], ps_rbc[:])
                    if h == H_LOC - 1 and not abl_no_outproj:
                        if b == 0:
                            dr = "act"
                        elif qi == NQ - 1:
                            dr = "tail"
                        else:
                            dr = "alt"
                        for tt in range(b * 16 + qi * 4, b * 16 + qi * 4 + 4):
                            outq.extend(outproj_steps(tt, dr))

                return {"s1": s1, "s2": s2}

            # ---- streamed attention (global exp->consume lag) ------------
            def attention_stream(blks, fill, prev, pre_hooks=None):
                assert pair_exp and sums_tree2 and s0_at >= stagger
                sts = [{"pts": [None] * NKT} for _ in blks]
                stages = {-1: prev}

                def st_exp(bi, kt):
                    if kt % 2 == 1:
                        return
                    b, h, qi = blks[bi]
                    qs = slice(b * S + qi * 512, b * S + (qi + 1) * 512)
                    st = sts[bi]
                    ks = slice(b * S + kt * P, b * S + (kt + 1) * P)
                    ks2 = slice(b * S + (kt + 1) * P, b * S + (kt + 2) * P)
                    ps_d = psp.tile([P, 1024], fp32, tag="psd", name="psd",
                                    bufs=2)
                    nc.tensor.matmul(
                        ps_d[:, 0:512], kt_sb[:, h, ks], qt_sb[:, h, qs],
                        start=True, stop=True,
                    )
                    nc.tensor.matmul(
                        ps_d[:, 512:1024], kt_sb[:, h, ks2], qt_sb[:, h, qs],
                        start=True, stop=True,
                    )
                    pt = ptp.tile([P, 1024], bf16, tag="pt", name="pt")
                    nc.scalar.activation(pt[:], ps_d[:], Exp, scale=SCALE)
                    st["pts"][kt] = (pt, 0)
                    st["pts"][kt + 1] = (pt, 512)

                def finalize(bi):
                    b, h, qi = blks[bi]
                    qs = slice(b * S + qi * 512, b * S + (qi + 1) * 512)
                    st = sts[bi]
                    ctxu = nrm.tile([P, 512], fp32, tag="ctxu", name="ctxu")
                    nc.vector.tensor_copy(ctxu[:], st["ctx"][:])
                    cur = {}
                    pair_tiles = [st["pts"][2 * i][0] for i in range(NKT // 2)]

                    def s0():
                        lvl = pair_tiles
                        li = 0
                        while len(lvl) > 1:
                            nxt = []
                            for i in range(0, len(lvl), 2):
                                t = accp.tile(
                                    [P, 1024], bf16, tag=f"tl{li}",
                                    name=f"tl{li}",
                                    bufs=(opts.get("tl0_bufs", 4)
                                          if li == 0 else 3),
                                )
                                nc.vector.tensor_add(
                                    t[:], lvl[i][:], lvl[i + 1][:]
                                )
                                nxt.append(t)
                            lvl = nxt
                            li += 1
                        accf2 = accp.tile([P, 512], bf16, tag="accs",
                                          name="accs", bufs=2)
                        nc.vector.tensor_add(
                            accf2[:], lvl[0][:, 0:512], lvl[0][:, 512:1024]
                        )
                        cur["accf"] = accf2

                    def s1():
                        ps_sum = ps_tile()
                        nc.tensor.matmul(
                            ps_sum[0:1, :], ones_sb[:], cur["accf"][:],
                            start=True, stop=True,
                        )
                        rcp = nrm.tile([1, 512], fp32, tag="rcp", name="rcp")
                        nc.vector.reciprocal(rcp[:], ps_sum[0:1, :])
                        rcpb = nrm.tile([1, 512], bf16, tag="rcpb",
                                        name="rcpb")
                        nc.vector.tensor_copy(rcpb[:], rcp[:])
                        cur["rcpb"] = rcpb

                    def s2():
                        ps_rbc = ps_tile()
                        nc.tensor.matmul(
                            ps_rbc[:], ones_row[:], cur["rcpb"][:],
                            start=True, stop=True,
                        )
                        nc.vector.tensor_mul(
                            ctx_sb[:, h, qs], ctxu[:], ps_rbc[:]
                        )
                        if h == H_LOC - 1 and not abl_no_outproj:
                            if b == 0:
                                dr = "act"
                            elif qi == NQ - 1:
                                dr = "tail"
                            else:
                                dr = "alt"
                            for tt in range(b * 16 + qi * 4,
                                            b * 16 + qi * 4 + 4):
                                outq.extend(outproj_steps(tt, dr))

                    stages[bi] = {"s0": s0, "s1": s1, "s2": s2}

                def consume(bi, kt):
                    b, h, qi = blks[bi]
                    hd = slice(h * P, (h + 1) * P)
                    st = sts[bi]
                    if kt == 0:
                        st["ctx"] = ps_tile()
                    pt, off = st["pts"][kt]
                    nc.tensor.matmul(
                        st["ctx"][:], v_sb[:, b * NKT + kt, hd],
                        pt[:, off:off + 512],
                        start=(kt == 0), stop=(kt == NKT - 1),
                    )
                    if kt == NKT - 1:
                        finalize(bi)

                seq = [(bi, kt) for bi in range(len(blks))
                       for kt in range(NKT)]
                for i, (bi, kt) in enumerate(seq):
                    if kt == 0 and pre_hooks and bi in pre_hooks:
                        pre_hooks[bi]()
                    st_exp(bi, kt)
                    pull_filler(fill)
                    pv = stages.get(bi - 1)
                    if pv:
                        if kt == s0_at and pv.get("s0"):
                            pv["s0"]()
                        if kt == s1_at and pv.get("s1"):
                            pv["s1"]()
                        if kt == s2_at and pv.get("s2"):
                            pv["s2"]()
                    j = i - stagger
                    if j >= 0:
                        consume(*seq[j])
                for j in range(len(seq) - stagger, len(seq)):
                    consume(*seq[j])
                return stages[len(blks) - 1]

            def finish(prev, spacing):
                if prev.get("s0"):
                    prev["s0"]()
                pull_filler(spacing)
                if prev.get("s1"):
                    prev["s1"]()
                pull_filler(spacing)
                if prev.get("s2"):
                    prev["s2"]()

            # ---- global schedule ----------------------------------------
            def emit_body():
                if abl_proj_only:
                    for c in range(4):
                        for s in chunk_steps(c, "all"):
                            s()
                    for c in range(4, CH):
                        for s in chunk_steps(c, "kv"):
                            s()
                    for c in range(4, CH):
                        for s in chunk_steps(c, "q"):
                            s()
                    return
                if abl_attn_only:
                    # fake producers for Q/K/V so attention has dependencies
                    nc.vector.memset(qt_sb[:], 0.001)
                    nc.vector.memset(kt_sb[:], 0.001)
                    nc.vector.memset(v_sb[:], 0.001)
                    nc.sync.dma_start(wo_sb[:], wot_v)
                    prev = {}
                    for b in range(B):
                        for qi in range(NQ):
                            for h in range(H_LOC):
                                prev = attention2(b, h, qi, 0, prev)
                    finish(prev, 2)
                    drain_oq()
                    return
                keys = [("a", c) for c in range(4)]
                if defer_q:
                    keys += [("kv", c) for c in range(4, CH)]
                    keys += [("q", c) for c in range(4, CH)]
                else:
                    keys += [("a", c) for c in range(4, CH)]
                for ka, kb in zip(keys, keys[1:]):
                    _next_key[ka] = kb
                for c in range(4):
                    for s in chunk_steps(c, "all"):
                        s()
                qpos = {}
                if defer_q:
                    for c in range(4, CH):
                        chunkq.extend(chunk_steps(c, "kv"))
                    for c in range(4, CH):
                        chunkq.extend(chunk_steps(c, "q"))
                        qpos[c] = len(chunkq)
                else:
                    for c in range(4, CH):
                        chunkq.extend(chunk_steps(c, "all"))
                    for c in range(4, CH):
                        qpos[c] = len(chunkq) if c >= 5 else 0
                    qpos[5] = len(chunkq)
                nc.sync.dma_start(wo_sb[:], wot_v)
                prev = {}
                if stream_attn:
                    b0_blks = [(0, h, qi) for qi in range(NQ)
                               for h in range(H_LOC)]
                    prev = attention_stream(b0_blks, fill_b0, prev)
                else:
                    for qi in range(NQ):
                        for h in range(H_LOC):
                            prev = attention2(0, h, qi, fill_b0, prev)
                finish(prev, 16)
                # drain phase: emit K/V for chunks 4-7 plus Q for chunks 4-5,
                # interleaving the b0 output projections (their psum drains
                # ride the idle ACT engine); Q for chunks 6-7 stays queued as
                # cheap filler for the early b1 blocks
                dstop = qpos[drain_to]
                while cqi[0] < dstop:
                    for _ in range(drain_ratio):
                        if cqi[0] < dstop:
                            chunkq[cqi[0]]()
                            cqi[0] += 1
                    if oqi[0] < len(outq):
                        outq[oqi[0]]()
                        oqi[0] += 1
                prev = {}
                if stream_attn:
                    b1_blks = [(1, h, qi) for qi in range(NQ)
                               for h in range(H_LOC)]
                    hooks = {
                        qi * H_LOC: (lambda q=qi: drain_cq_to(qpos[4 + q]))
                        for qi in range(NQ)
                    }
                    prev = attention_stream(b1_blks, fill_b1, prev,
                                            pre_hooks=hooks)
                else:
                    for qi in range(NQ):
                        drain_cq_to(qpos[4 + qi])  # this qi's Q must be emitted
                        for h in range(H_LOC):
                            last = qi == NQ - 1 and h == H_LOC - 1
                            prev = attention2(
                                1, h, qi, fill_b1, prev,
                                stg=(4 if attn_batch4 else 2) if last else None,
                            )
                finish(prev, 2)
                drain_cq_to(len(chunkq))
                drain_oq()

            if loop_k is None:
                emit_body()
            else:
                with tc.For_i(0, loop_k, 1):
                    emit_body()

    _split_multi_waits(nc)
    return nc


# Final tuned configuration (HW-validated via slope benchmarking + cost model):
#  - out_bf16: bf16 partial outputs (halves store traffic; host sums in fp64)
#  - obp_bufs=8: deep store pipeline (phase-3 tail was store-bound)
#  - split_in: split startup DMAs so the first matmuls start early
#  - norm2: drain ctx psum early; reciprocal broadcast off the critical path
#  - act_lite: ACT engine reserved for exps; bias-adds/drains on DVE
#  - sched2 + stagger: interleave projection/attention/out-projection emission
#    and software-pipeline the attention loop so exp latency never stalls PE
_DEFAULT_OPTS = dict(
    out_bf16=True, obp_bufs=8, split_in=True, norm2=True,
    act_lite=True, sched2=True, stagger=2, early_x=True,
)

# v2 schedule (see _build_nc2): DVE-accumulated softmax sums + fine-grained
# filler interleave. Defaults here are the sim-tuned configuration.
_DEFAULT_OPTS2 = dict()
# With an all-zero attention mask the per-key exp bias is unused, so each
# pair of score tiles can share one [128,1024] exp (HW A/B: ~3% faster).
# alt_mod=2: with paired exps ACT has slack, so b1 outproj drains split
# evenly between DVE and ACT (HW A/B: slightly faster than 3-of-4 on DVE).
# v3 addition (HW paired A/B, 2026-08-11): sums_tree2 — softmax
# denominators via a DVE adder tree over the block's retained exp tiles,
# deferred into the next block (s0), replacing the per-exp accumulate
# chain. The chain serialized DVE behind each fresh exp (ACT->DVE sem
# coupling + queue head-of-line blocking of the outproj drains); the tree
# runs on long-ready inputs. Paired A/B: -17us vs the v2 chain.
# (attn_batch4 / norm_dma / sums_pe / stream_attn were also built and
# HW-benched; none beat plain sums_tree2 — see bench logs + memory notes.)
_PAIR_OPTS = dict(
    pair_exp=True, pt_bufs=11, xch_bufs=3, tl0_bufs=4, obp_bufs=8,
    ps_bufs=4, alt_mod=2, drain_to=4, drain_ratio=2,
    sums_tree2=True, s0_at=1, s1_at=6, s2_at=12,
    # fill_b0=2: pull two filler matmuls per kt in the b0 blocks (paired
    # A/B -3us; shortens the post-b0 drain phase more than it stretches
    # the ACT-slack-limited b0 blocks)
    fill_b0=2,
)


def _get_nc(zero_mask=False):
    key = ("nc", zero_mask)
    if key not in _CACHE:
        if os.environ.get("MHA_KERNEL_V", "2") == "1":
            _CACHE[key] = _build_nc(**_DEFAULT_OPTS)
        else:
            opts = dict(_DEFAULT_OPTS2)
            if zero_mask:
                opts.update(_PAIR_OPTS)
            _CACHE[key] = _build_nc2(**opts)
    return _CACHE[key]


def kernel(**inputs):
    hs = np.asarray(inputs["hidden_states"], dtype=np.float32)
    mask = np.asarray(inputs["attention_mask"], dtype=np.float32)
    Wq = np.asarray(inputs["Wq"], dtype=np.float32)
    bq = np.asarray(inputs["bq"], dtype=np.float32)
    Wk = np.asarray(inputs["Wk"], dtype=np.float32)
    bk = np.asarray(inputs["bk"], dtype=np.float32)
    Wv = np.asarray(inputs["Wv"], dtype=np.float32)
    bv = np.asarray(inputs["bv"], dtype=np.float32)
    Wo = np.asarray(inputs["Wo"], dtype=np.float32)
    bo = np.asarray(inputs["bo"], dtype=np.float32)

    x = hs.reshape(T, HIDDEN)
    xt = np.ascontiguousarray(x.T).astype(BF16NP)
    mask2 = np.ascontiguousarray(mask.reshape(B, S))

    in_maps = []
    for c in range(N_CORES):
        rs = slice(c * DLOC, (c + 1) * DLOC)
        in_maps.append({
            "xt": xt,
            "wqt": np.ascontiguousarray(Wq[rs, :].T).astype(BF16NP),
            "wkt": np.ascontiguousarray(Wk[rs, :].T).astype(BF16NP),
            "wvt": np.ascontiguousarray(Wv[rs, :].T).astype(BF16NP),
            "wot": np.ascontiguousarray(Wo[:, rs].T).astype(BF16NP),
            "bq": np.ascontiguousarray(bq[rs]),
            "bk": np.ascontiguousarray(bk[rs]),
            "mask": mask2,
        })

    from concourse.bass_utils import run_bass_kernel_spmd

    nc = _get_nc(zero_mask=not np.any(mask2))
    trace = bool(int(os.environ.get("MHA_KERNEL_TRACE", "0")))

    def _run():
        return run_bass_kernel_spmd(
            nc, in_maps, core_ids=list(range(N_CORES)), trace=trace,
            **({"trace_cores": list(range(N_CORES))} if trace else {}),
        )

    try:
        res = _run()
    except Exception:
        # transient device errors (e.g. NRT_EXEC_UNIT_UNRECOVERABLE after a
        # prior process wedged the core) have been observed to clear on a
        # retry; one retry costs nothing on the success path
        res = _run()
    _CACHE["last_results"] = res

    out = np.sum(
        np.stack([r["out"] for r in res.results]), axis=0, dtype=np.float64
    )
    out += bv.astype(np.float64) @ Wo.T.astype(np.float64) + bo
    return out.astype(np.float32).reshape(B, S, HIDDEN)



# revision 33
# speedup vs baseline: 1.0076x; 1.0076x over previous
"""Multi-head attention forward (B=2, S=2048, H=2048, 16 heads) on 8 TRN2 NeuronCores.

Sharding: tensor-parallel over heads — 2 heads per core. Each core computes
Q/K/V projections for its 2 heads (full batch), attention, and a partial
output projection (its heads' columns of Wo); the host sums the 8 partial
outputs and adds the bias terms.

Device compute is bf16 with fp32 PSUM accumulation. Host pre-transposes
the activation matrix (X.T) and weight slices so the device never has to
transpose fp32 data (fp32 DMA transpose is unsupported).

Layout notes (matmul computes lhsT.T @ rhs, contracting the partition dim):
  - Q.T, K.T are computed as [head_dim, tokens] (d on partitions):
        lhsT = Wq.T tile [hid, d], rhs = X.T tile [hid, tokens]
  - V is computed natural [tokens, d]: lhsT = X.T tile, rhs = Wv.T tile
  - scores transposed S.T[k_tok, q] = (K.T tile).T @ Q.T  (contract d=128)
  - P.T = exp(SCALE * S.T + mask) via one scalar-engine activation
    (mask is per-key = per-partition, so it rides the activation bias;
    with an all-zero mask, two score tiles share one [128,1024] exp)
  - ctx.T[d, q] = V_tile.T @ P.T (contract k_tok), accumulated over k tiles
  - softmax denominators accumulate on DVE (bf16 accumulator tiles
    trailing the exps) with one ones-vector matmul per attention block
  - 1/sums broadcast across partitions via a ones-row matmul (norm tail
    deferred into the next block so no engine ever waits on the chain)
  - out_partial[t, o] = (ctx.T tile).T @ Wo.T tile (contract local head dims)

Schedule (_build_nc2): the PE instruction stream is a single sequence in
which projection and output-projection matmuls are interleaved at key-tile
granularity inside the attention blocks ("filler"), so the tensor engine
never waits on the scalar engine's exps. Emission order: chunks 0-3 (batch
0 projections) -> b0 attention blocks (qi-major, chunk 4-7 filler) -> drain
remaining chunk work interleaved with b0 output projections (psum drains on
the otherwise idle ACT engine) -> b1 attention blocks (outproj filler) ->
tail. PE busy is within ~2% of the bf16 matmul floor for this layout.

bv/bo are folded on the host: rows of normalized P sum to 1, so
ctx = P@(V + bv) = P@V + bv, giving out += bv @ Wo.T + bo after the
cross-core reduction.

Measured (same-session A/B slope benchmarking, bench.py): v2 pair_exp
~462us/iter vs the v1 schedule ~554us/iter on the same device state
(the graded single-shot baseline for v1 was 502897ns).

v3 (2026-08-11): softmax denominators moved from the per-exp DVE
accumulate chain to a deferred DVE adder tree over the block's retained
exp tiles (sums_tree2, emitted at the next block's start). The chain's
per-op waits on fresh ACT exps serialized the DVE queue and blocked the
outproj drains queued behind it; the tree's inputs are all long-ready.
Paired interleaved A/B (14 rounds, K=129 in-NEFF repeats): -17us vs v2
on the same device state. HW microbenchmarks (micro.py): 512-wide bf16
MM 238-250ns (vs 216 ideal; +25ns/MM per extra interleaved PSUM-bank
accumulation group), exp[128,1024] PSUM->SBUF 976ns, DVE add[128,1024]
bf16 678ns (2x mode does not engage). Full-kernel PE busy is ~380us at
these rates, so the schedule is within ~8-14% of this algorithm's
per-instruction PE floor. Also built and HW-rejected: per-kt PE
ones-matmul sums (+90us: ldweights alternation + a PE->DVE->PE psum-bank
cycle), cross-block exp/consume streaming (pt-pool pressure, no win),
4-kt super-step batching and DMA-broadcast norm (no replicated win over
plain sums_tree2).
"""

import os

import numpy as np
import ml_dtypes

P = 128
HIDDEN = 2048
NUM_HEADS = 16
HEAD_DIM = 128
B, S = 2, 2048
T = B * S                     # 4096 tokens
N_CORES = 8
H_LOC = NUM_HEADS // N_CORES  # 2 heads per core
DLOC = H_LOC * HEAD_DIM       # 256
KO = HIDDEN // P              # 16 contraction tiles for the projections
CH = 8                        # token chunks for the projection phase
CHW = T // CH                 # 512 tokens per chunk
NKT = S // P                  # 16 key tiles per batch
NQ = S // 512                 # 4 query tiles (512 wide) per batch
SCALE = float(1.0 / np.sqrt(HEAD_DIM).astype(np.float32))

BF16NP = ml_dtypes.bfloat16

_CACHE = {}


def _split_multi_waits(nc):
    """Split instructions carrying >1 semaphore wait.

    This walrus build rejects any instruction with more than one sync wait
    ("Too many sync wait commands"), but Tile's wait assignment freely
    attaches several. Hoist all but the last wait onto same-engine NOPs
    inserted immediately before the instruction — each engine sequencer
    executes its queue in order, so blocking on a preceding NOP is
    equivalent to blocking on the instruction itself.
    """
    import bass_rust
    import concourse.mybir as mybir

    cnt = 0
    for f in nc.m.functions:
        for bb in f.blocks:
            out = []
            for inst in bb.instructions:
                si = inst.sync_info
                waits = list(si.on_wait) if si and si.on_wait else []
                if len(waits) > 1:
                    for w in waits[:-1]:
                        nop = mybir.InstNoOp(name=f"wsplit_{cnt}", ins=[], outs=[])
                        cnt += 1
                        nop.engine = inst.engine
                        nop.sync_info = bass_rust.SyncInfo(on_wait=[w], on_update=[])
                        out.append(nop)
                    inst.sync_info = bass_rust.SyncInfo(
                        on_wait=[waits[-1]], on_update=list(si.on_update or [])
                    )
                out.append(inst)
            bb.instructions[:] = out
    return cnt


def _build_nc(loop_k=None, **opts):
    """Build the kernel module.

    loop_k: if set, wrap the whole compute body in a For_i running it loop_k
    times — used only for benchmarking (slope timing); the graded kernel
    uses loop_k=None (straight-line body).
    opts: benchmark-only ablation switches (default: all off).
    """
    import concourse.bass as bass
    import concourse.mybir as mybir
    import concourse.tile as tile

    no_sums = opts.get("no_sums", False)
    no_phase3 = opts.get("no_phase3", False)
    no_out_dma = opts.get("no_out_dma", False)
    no_attn = opts.get("no_attn", False)
    xch_bufs = opts.get("xch_bufs", 3)
    pt_bufs = opts.get("pt_bufs", 4)
    norm2 = opts.get("norm2", False)        # deferred norm (early psum drain)
    interleave = opts.get("interleave", False)  # phase 2/3 interleaved per batch
    fuse = opts.get("fuse", False)          # phase 2/3 fused at qi granularity
    vcopy_act = opts.get("vcopy_act", False)  # V psum drain on scalar engine
    norm_gp = opts.get("norm_gp", False)    # norm bounce DMAs on ACT queues
    xt_gp = opts.get("xt_gp", False)        # xt streaming loads on ACT queues
    norm3 = opts.get("norm3", False)        # reciprocal broadcast via PE matmul
    sched2 = opts.get("sched2", False)      # global proj/attn/outproj interleave
    stagger = opts.get("stagger", 0)        # ctx MM issued N steps behind S.T
    act_lite = opts.get("act_lite", False)  # keep ACT for exps only
    early_x = opts.get("early_x", False)    # first x chunk loads before consts
    sums_defer = opts.get("sums_defer", False)  # sums MMs after the kt loop
    sums_tree = opts.get("sums_tree", None)  # "gpsimd"|"vector": adder tree
    sums_acc = opts.get("sums_acc", None)   # "gpsimd"|"vector": inline accum
    ps_bufs = opts.get("ps_bufs", 8)
    out_bf16 = opts.get("out_bf16", False)  # bf16 partial output
    obp_bufs = opts.get("obp_bufs", 3)
    split_in = opts.get("split_in", False)  # split startup DMAs for fast ramp

    fp32 = mybir.dt.float32
    bf16 = mybir.dt.bfloat16

    nc = bass.Bass()

    xt_d = nc.dram_tensor("xt", [HIDDEN, T], bf16, kind="ExternalInput")
    wqt_d = nc.dram_tensor("wqt", [HIDDEN, DLOC], bf16, kind="ExternalInput")
    wkt_d = nc.dram_tensor("wkt", [HIDDEN, DLOC], bf16, kind="ExternalInput")
    wvt_d = nc.dram_tensor("wvt", [HIDDEN, DLOC], bf16, kind="ExternalInput")
    wot_d = nc.dram_tensor("wot", [DLOC, HIDDEN], bf16, kind="ExternalInput")
    bq_d = nc.dram_tensor("bq", [DLOC], fp32, kind="ExternalInput")
    bk_d = nc.dram_tensor("bk", [DLOC], fp32, kind="ExternalInput")
    mask_d = nc.dram_tensor("mask", [B, S], fp32, kind="ExternalInput")
    out_dt = bf16 if out_bf16 else fp32
    out_d = nc.dram_tensor("out", [T, HIDDEN], out_dt, kind="ExternalOutput")

    xt_v = xt_d[:].rearrange("(ko p) t -> p ko t", p=P)
    wqt_v = wqt_d[:].rearrange("(ko p) d -> p ko d", p=P)
    wkt_v = wkt_d[:].rearrange("(ko p) d -> p ko d", p=P)
    wvt_v = wvt_d[:].rearrange("(ko p) d -> p ko d", p=P)
    wot_v = wot_d[:].rearrange("(h p) o -> p h o", p=P)
    bq_v = bq_d[:].rearrange("(h p) -> p h", p=P)
    bk_v = bk_d[:].rearrange("(h p) -> p h", p=P)
    mask_v = mask_d[:].rearrange("b (ko p) -> p b ko", p=P)

    with tile.TileContext(nc) as tc:
        with (
            tc.tile_pool(name="const", bufs=1) as const,
            tc.tile_pool(name="big", bufs=1) as big,
            tc.tile_pool(name="xch", bufs=xch_bufs) as xch,
            tc.tile_pool(name="ptp", bufs=pt_bufs) as ptp,
            tc.tile_pool(name="nrm", bufs=opts.get("nrm_bufs", 2)) as nrm,
            tc.tile_pool(name="ob", bufs=obp_bufs) as obp,
            tc.tile_pool(name="ps", bufs=ps_bufs, space="PSUM") as psp,
            tc.tile_pool(name="dscr", bufs=4, space="DRAM") as dscr,
        ):
            Ident = mybir.ActivationFunctionType.Identity
            Exp = mybir.ActivationFunctionType.Exp

            def ps_tile():
                return psp.tile([P, 512], fp32, tag="ps", name="ps")

            # ---- resident constants -------------------------------------
            wq_sb = const.tile([P, KO, DLOC], bf16)
            wk_sb = const.tile([P, KO, DLOC], bf16)
            wv_sb = const.tile([P, KO, DLOC], bf16)
            wo_sb = const.tile([P, H_LOC, HIDDEN], bf16)
            bq_sb = const.tile([P, H_LOC], fp32)
            bk_sb = const.tile([P, H_LOC], fp32)
            mask_sb = const.tile([P, B, NKT], fp32)
            ones_sb = const.tile([P, 1], bf16)
            ones_row = const.tile([1, P], bf16)   # lhsT for rcp broadcast MM

            xc0 = None
            if split_in:
                for ko in range(0, KO, 4):
                    nc.sync.dma_start(wq_sb[:, ko:ko + 4, :], wqt_v[:, ko:ko + 4, :])
                if early_x:
                    # queue the first activation chunk ahead of the remaining
                    # constants so the first projection matmuls start early
                    xc0 = xch.tile([P, KO, CHW], bf16, tag="xc", name="xc")
                    for ko in range(0, KO, 4):
                        nc.sync.dma_start(
                            xc0[:, ko:ko + 4, :], xt_v[:, ko:ko + 4, 0:CHW]
                        )
                for ko in range(0, KO, 4):
                    nc.sync.dma_start(wk_sb[:, ko:ko + 4, :], wkt_v[:, ko:ko + 4, :])
                for ko in range(0, KO, 4):
                    nc.sync.dma_start(wv_sb[:, ko:ko + 4, :], wvt_v[:, ko:ko + 4, :])
            else:
                nc.sync.dma_start(wq_sb[:], wqt_v)
                nc.sync.dma_start(wk_sb[:], wkt_v)
                nc.sync.dma_start(wv_sb[:], wvt_v)
            nc.sync.dma_start(wo_sb[:], wot_v)
            nc.sync.dma_start(bq_sb[:], bq_v)
            nc.sync.dma_start(bk_sb[:], bk_v)
            nc.sync.dma_start(mask_sb[:], mask_v)
            nc.vector.memset(ones_sb[:], 1.0)
            nc.vector.memset(ones_row[:], 1.0)

            # ---- big activation buffers ---------------------------------
            qt_sb = big.tile([P, H_LOC, T], bf16)   # Q.T  (d on partitions)
            kt_sb = big.tile([P, H_LOC, T], bf16)   # K.T
            v_sb = big.tile([P, T // P, DLOC], bf16)  # V natural (t on partitions)
            ctx_sb = big.tile([P, H_LOC, T], bf16)  # ctx.T

            def emit_body():
                if sched2:
                    # Global interleave: keep ACT-independent matmul work
                    # (projections / output projection) flowing between
                    # attention blocks so exp latency never stalls PE.
                    emit_phase1(range(0, 4))
                    att_b0 = [(0, h, qi) for h in range(H_LOC) for qi in range(NQ)]
                    for i, c in enumerate(range(4, CH)):
                        emit_phase1([c])
                        for blk in att_b0[2 * i:2 * i + 2]:
                            attention(*blk)
                    att_b1 = [(1, h, qi) for h in range(H_LOC) for qi in range(NQ)]
                    for i, blk in enumerate(att_b1):
                        attention(*blk)
                        if not no_phase3:
                            outproj(2 * i, copy_eng=0)
                            outproj(2 * i + 1, copy_eng=1)
                    if not no_phase3:
                        for tt in range(T // P // 2, T // P):
                            outproj(tt, copy_eng=tt % 2)
                    return
                emit_phase1()
                if fuse:
                    # qi-granular fusion: as soon as both heads of a q-tile
                    # are done, run its output projection + store.
                    for b in range(B):
                        for qi in range(NQ):
                            for h in range(H_LOC):
                                attention(b, h, qi)
                            if not no_phase3:
                                for j in range(4):
                                    outproj(b * 16 + qi * 4 + j, copy_eng=j % 2)
                elif interleave:
                    emit_phase2([0])
                    emit_phase3(range(0, T // P // 2))
                    emit_phase2([1])
                    emit_phase3(range(T // P // 2, T // P))
                else:
                    emit_phase2()
                    emit_phase3()

            # ---- phase 1: Q/K/V projections, streamed over token chunks --
            def emit_phase1(cs=tuple(range(CH))):
              for c in cs:
                if c == 0 and xc0 is not None:
                    xc = xc0
                else:
                    xc = xch.tile([P, KO, CHW], bf16, tag="xc", name="xc")
                    xt_eng = nc.scalar if xt_gp else nc.sync
                    if split_in:
                        for ko in range(0, KO, 4):
                            xt_eng.dma_start(
                                xc[:, ko:ko + 4, :],
                                xt_v[:, ko:ko + 4, c * CHW:(c + 1) * CHW],
                            )
                    else:
                        xt_eng.dma_start(xc[:], xt_v[:, :, c * CHW:(c + 1) * CHW])

                for h in range(H_LOC):
                    hd = slice(h * P, (h + 1) * P)
                    psq = ps_tile()
                    for ko in range(KO):
                        nc.tensor.matmul(
                            psq[:], wq_sb[:, ko, hd], xc[:, ko, :],
                            start=(ko == 0), stop=(ko == KO - 1),
                        )
                    if act_lite:
                        nc.vector.tensor_scalar_add(
                            qt_sb[:, h, c * CHW:(c + 1) * CHW], psq[:],
                            bq_sb[:, h:h + 1],
                        )
                    else:
                        nc.scalar.activation(
                            qt_sb[:, h, c * CHW:(c + 1) * CHW], psq[:],
                            Ident, bias=bq_sb[:, h:h + 1],
                        )
                    psk = ps_tile()
                    for ko in range(KO):
                        nc.tensor.matmul(
                            psk[:], wk_sb[:, ko, hd], xc[:, ko, :],
                            start=(ko == 0), stop=(ko == KO - 1),
                        )
                    if act_lite:
                        nc.vector.tensor_scalar_add(
                            kt_sb[:, h, c * CHW:(c + 1) * CHW], psk[:],
                            bk_sb[:, h:h + 1],
                        )
                    else:
                        nc.scalar.activation(
                            kt_sb[:, h, c * CHW:(c + 1) * CHW], psk[:],
                            Ident, bias=bk_sb[:, h:h + 1],
                        )

                for tt in range(CHW // P):
                    psv = ps_tile()
                    for ko in range(KO):
                        nc.tensor.matmul(
                            psv[:, :DLOC], xc[:, ko, tt * P:(tt + 1) * P],
                            wv_sb[:, ko, :],
                            start=(ko == 0), stop=(ko == KO - 1),
                        )
                    if vcopy_act:
                        nc.scalar.copy(v_sb[:, c * (CHW // P) + tt, :], psv[:, :DLOC])
                    else:
                        nc.vector.tensor_copy(
                            v_sb[:, c * (CHW // P) + tt, :], psv[:, :DLOC]
                        )

            # ---- phase 2: attention for one (batch, head, q-tile) --------
            def attention(b, h, qi):
                hd = slice(h * P, (h + 1) * P)
                qs = slice(b * S + qi * 512, b * S + (qi + 1) * 512)
                ps_ctx = ps_tile()
                ps_sum = ps_tile()
                pts = []
                accs = [None, None]
                aeng = None
                if sums_acc is not None:
                    aeng = nc.gpsimd if sums_acc == "gpsimd" else nc.vector
                def emit_st_exp(kt):
                    ks = slice(b * S + kt * P, b * S + (kt + 1) * P)
                    ps_s = ps_tile()
                    nc.tensor.matmul(
                        ps_s[:], kt_sb[:, h, ks], qt_sb[:, h, qs],
                        start=True, stop=True,
                    )
                    pt = ptp.tile([P, 512], bf16, tag="pt", name="pt")
                    nc.scalar.activation(
                        pt[:], ps_s[:], Exp,
                        bias=mask_sb[:, b, kt:kt + 1], scale=SCALE,
                    )
                    pts.append(pt)

                def emit_consume(kt):
                    pt = pts[kt]
                    nc.tensor.matmul(
                        ps_ctx[:], v_sb[:, b * NKT + kt, hd], pt[:],
                        start=(kt == 0), stop=(kt == NKT - 1),
                    )
                    if no_sums:
                        return
                    if sums_acc is not None:
                        # two interleaved accumulators trail the exps
                        i = kt % 2
                        if accs[i] is None:
                            accs[i] = ptp.tile(
                                [P, 512], bf16, tag=f"sacc{i}",
                                name=f"sacc{i}", bufs=2,
                            )
                            aeng.tensor_copy(accs[i][:], pt[:])
                        else:
                            aeng.tensor_add(accs[i][:], accs[i][:], pt[:])
                    elif not sums_defer and sums_tree is None:
                        nc.tensor.matmul(
                            ps_sum[0:1, :], ones_sb[:], pt[:],
                            start=(kt == 0), stop=(kt == NKT - 1),
                        )

                for kt in range(NKT):
                    emit_st_exp(kt)
                    if not no_attn and kt >= stagger:
                        emit_consume(kt - stagger)
                if not no_attn:
                    for kt in range(NKT - stagger, NKT):
                        emit_consume(kt)
                if no_attn:
                    return
                if sums_acc is not None and not no_sums:
                    aeng.tensor_add(accs[0][:], accs[0][:], accs[1][:])
                    nc.tensor.matmul(
                        ps_sum[0:1, :], ones_sb[:], accs[0][:],
                        start=True, stop=True,
                    )
                if sums_defer and not no_sums:
                    for kt in range(NKT):
                        nc.tensor.matmul(
                            ps_sum[0:1, :], ones_sb[:], pts[kt][:],
                            start=(kt == 0), stop=(kt == NKT - 1),
                        )
                if sums_tree is not None and not no_sums:
                    # Pairwise-add the 16 exp tiles on a non-PE engine, then a
                    # single ones-matmul does the partition reduction.
                    teng = nc.gpsimd if sums_tree == "gpsimd" else nc.vector
                    lvl = list(pts)
                    li = 0
                    while len(lvl) > 1:
                        nxt = []
                        for i in range(0, len(lvl), 2):
                            t = ptp.tile(
                                [P, 512], bf16, tag=f"tl{li}", name=f"tl{li}",
                                bufs=(10 if li == 0 else 5),
                            )
                            teng.tensor_add(t[:], lvl[i][:], lvl[i + 1][:])
                            nxt.append(t)
                        lvl = nxt
                        li += 1
                    nc.tensor.matmul(
                        ps_sum[0:1, :], ones_sb[:], lvl[0][:],
                        start=True, stop=True,
                    )
                if no_sums:
                    nc.vector.tensor_copy(ctx_sb[:, h, qs], ps_ctx[:])
                    return
                rcp = nrm.tile([1, 512], fp32, tag="rcp", name="rcp")
                nc.vector.reciprocal(rcp[:], ps_sum[0:1, :])
                if norm3:
                    # Broadcast 1/sums across partitions with one K=1 matmul
                    # (ones_row.T @ rcp) — no DMA round trip on the critical
                    # path to ctx_sb.
                    rcpb = nrm.tile([1, 512], bf16, tag="rcpb", name="rcpb")
                    nc.vector.tensor_copy(rcpb[:], rcp[:])
                    ps_rbc = ps_tile()
                    nc.tensor.matmul(
                        ps_rbc[:], ones_row[:], rcpb[:], start=True, stop=True,
                    )
                    ctxu = nrm.tile([P, 512], fp32, tag="ctxu", name="ctxu")
                    nc.vector.tensor_copy(ctxu[:], ps_ctx[:])
                    nc.vector.tensor_mul(ctx_sb[:, h, qs], ctxu[:], ps_rbc[:])
                    return
                rbc = nrm.tile([P, 512], fp32, tag="rbc", name="rbc")
                rdr = dscr.tile([1, 512], fp32, tag="rdr", name="rdr")
                dma_eng = nc.scalar if norm_gp else nc.sync
                if norm2:
                    # Drain the ctx psum to SBUF right away (frees the
                    # bank); the reciprocal broadcast (DRAM bounce)
                    # happens off the critical path.
                    ctxu = nrm.tile([P, 512], fp32, tag="ctxu", name="ctxu")
                    nc.vector.tensor_copy(ctxu[:], ps_ctx[:])
                    dma_eng.dma_start(rdr[:], rcp[:])
                    dma_eng.dma_start(rbc[:], rdr[:].to_broadcast((P, 512)))
                    nc.vector.tensor_mul(ctx_sb[:, h, qs], ctxu[:], rbc[:])
                else:
                    dma_eng.dma_start(rdr[:], rcp[:])
                    dma_eng.dma_start(rbc[:], rdr[:].to_broadcast((P, 512)))
                    nc.vector.tensor_mul(ctx_sb[:, h, qs], ps_ctx[:], rbc[:])

            def emit_phase2(bs=tuple(range(B))):
                for b in bs:
                    for h in range(H_LOC):
                        for qi in range(NQ):
                            attention(b, h, qi)

            # ---- phase 3: partial output projection ----------------------
            def outproj(tt, copy_eng=0):
                ts_ = slice(tt * P, (tt + 1) * P)
                for oi in range(HIDDEN // 512):
                    os_ = slice(oi * 512, (oi + 1) * 512)
                    ps_o = ps_tile()
                    for h in range(H_LOC):
                        nc.tensor.matmul(
                            ps_o[:], ctx_sb[:, h, ts_], wo_sb[:, h, os_],
                            start=(h == 0), stop=(h == H_LOC - 1),
                        )
                    ob = obp.tile([P, 512], out_dt, tag="ob", name="ob")
                    if not act_lite and (copy_eng + oi) % 2:
                        nc.scalar.copy(ob[:], ps_o[:])
                    else:
                        nc.vector.tensor_copy(ob[:], ps_o[:])
                    if not no_out_dma:
                        nc.sync.dma_start(out_d[ts_, os_], ob[:])

            def emit_phase3(tts=tuple(range(T // P))):
                if no_phase3:
                    return
                for tt in tts:
                    outproj(tt)

            if loop_k is None:
                emit_body()
            else:
                with tc.For_i(0, loop_k, 1):
                    emit_body()

    _split_multi_waits(nc)
    return nc


def _build_nc2(loop_k=None, **opts):
    """v2 schedule.

    Differences from v1:
      - softmax denominators accumulate on DVE (two interleaved bf16
        accumulators trailing the exps) with a single ones-matmul per
        block, removing ~51us of PE ones-matmul work;
      - projection (chunks 4-7) and output-projection matmuls are fed
        into the attention blocks as per-kt filler so PE keeps streaming
        while ACT produces exps;
      - b0/b1 attention blocks run qi-major and output projections are
        appended as soon as both heads of a q-tile are normalized, which
        spreads the store traffic and shrinks the tail.
    """
    import concourse.bass as bass
    import concourse.mybir as mybir
    import concourse.tile as tile

    stagger = opts.get("stagger", 6)
    fill_b0 = opts.get("fill_b0", 1)
    fill_b1 = opts.get("fill_b1", 2)
    xch_bufs = opts.get("xch_bufs", 3)
    pt_bufs = opts.get("pt_bufs", 14)
    ps_bufs = opts.get("ps_bufs", 8)
    obp_bufs = opts.get("obp_bufs", 16)
    acc_bufs = opts.get("acc_bufs", 2)
    s1_at = opts.get("s1_at", 6)
    s2_at = opts.get("s2_at", 10)
    norm_dma = opts.get("norm_dma", False)
    norm_pool = opts.get("norm_pool", False)  # denominator reduce on gpsimd
    # pair_exp: fuse each pair of score tiles into one [128,1024] exp (double
    # psum bank read). Only valid when the attention mask is all-zero (the
    # per-key bias column differs between the two tiles otherwise); kernel()
    # selects it at build time after inspecting the mask.
    pair_exp = opts.get("pair_exp", False)
    # defer_q: split chunks 4-7 into K/V and Q passes, keeping Q of chunks
    # 6-7 as cheap filler for the early b1 attention blocks
    defer_q = opts.get("defer_q", True)
    # drain_to: which chunk's Q pass the pre-b1 drain phase runs through
    # (later Q passes become b1-block filler, pulled under emission guards)
    drain_to = opts.get("drain_to", 5)
    tail_split = opts.get("tail_split", False)  # half-tile tail drains/stores
    drain_ratio = opts.get("drain_ratio", 3)
    alt_mod = opts.get("alt_mod", 4)  # 1 of alt_mod b1 drains goes to ACT
    # NOTE: gpsimd cannot read PSUM in this toolchain (compile fails), so
    # outproj drains must stay on ACT/DVE.
    b1_pool = opts.get("b1_pool", False)
    out_bf16 = opts.get("out_bf16", True)
    # bench-only ablations (never set by kernel(); numerics are wrong):
    abl_no_store = opts.get("abl_no_store", False)   # skip out DMA stores
    abl_no_xt = opts.get("abl_no_xt", False)         # chunks>0 reuse xc0
    abl_half_exp = opts.get("abl_half_exp", False)   # alias every other exp
    abl_no_outproj = opts.get("abl_no_outproj", False)
    abl_proj_only = opts.get("abl_proj_only", False)  # phase 1 only
    abl_attn_only = opts.get("abl_attn_only", False)  # phase 2 only (memset QKV)
    abl_no_acc = opts.get("abl_no_acc", False)   # no DVE sums, raw ctx copy
    abl_raw_ctx = opts.get("abl_raw_ctx", False)  # sums kept, no norm mul
    acc_pool = opts.get("acc_pool", False)  # acc chain on gpsimd (correct)
    # sums_pe: softmax denominators via per-kt ones-matmuls accumulating in
    # a dedicated PSUM bank (PE-local chain after each ctx matmul), instead
    # of the DVE accumulate chain. Removes the ACT->DVE sem coupling that
    # stalls the DVE queue (and everything behind it) on HW.
    sums_pe = opts.get("sums_pe", False)
    # sums_tree2: keep all 8 exp pair-tiles of a block alive and reduce them
    # with a deferred DVE adder tree at the START of the next block (s0),
    # followed by one ones-matmul (s1). Same DVE op count as the chain but
    # all inputs are long-ready: no per-op ACT wait, no queue head-of-line.
    sums_tree2 = opts.get("sums_tree2", False)
    s0_at = opts.get("s0_at", 1)
    # stream_attn: software-pipeline attention ACROSS blocks — the score/exp
    # stream runs a GLOBAL `stagger` steps ahead of the ctx-consume stream,
    # so block seams have no consume-tail during which ACT starves.
    # Requires pair_exp + sums_tree2, and s0_at >= stagger (a block's
    # finalize only exists once its last consume has been emitted).
    stream_attn = opts.get("stream_attn", False)
    # attn_batch4: emit attention in 4-kt super-steps — 4 score MMs, then 4
    # consecutive ctx MMs, then a filler burst — so PE runs long same-bank
    # stretches. HW charges ~+25ns/MM per additional interleaved PSUM bank
    # (mm_alt2/mm_alt3 microbench), so fine-grained interleave is costly.
    # Requires pair_exp; stagger must be a multiple of 4.
    attn_batch4 = opts.get("attn_batch4", False)

    fp32 = mybir.dt.float32
    bf16 = mybir.dt.bfloat16

    nc = bass.Bass()

    xt_d = nc.dram_tensor("xt", [HIDDEN, T], bf16, kind="ExternalInput")
    wqt_d = nc.dram_tensor("wqt", [HIDDEN, DLOC], bf16, kind="ExternalInput")
    wkt_d = nc.dram_tensor("wkt", [HIDDEN, DLOC], bf16, kind="ExternalInput")
    wvt_d = nc.dram_tensor("wvt", [HIDDEN, DLOC], bf16, kind="ExternalInput")
    wot_d = nc.dram_tensor("wot", [DLOC, HIDDEN], bf16, kind="ExternalInput")
    bq_d = nc.dram_tensor("bq", [DLOC], fp32, kind="ExternalInput")
    bk_d = nc.dram_tensor("bk", [DLOC], fp32, kind="ExternalInput")
    mask_d = nc.dram_tensor("mask", [B, S], fp32, kind="ExternalInput")
    out_dt = bf16 if out_bf16 else fp32
    out_d = nc.dram_tensor("out", [T, HIDDEN], out_dt, kind="ExternalOutput")

    xt_v = xt_d[:].rearrange("(ko p) t -> p ko t", p=P)
    wqt_v = wqt_d[:].rearrange("(ko p) d -> p ko d", p=P)
    wkt_v = wkt_d[:].rearrange("(ko p) d -> p ko d", p=P)
    wvt_v = wvt_d[:].rearrange("(ko p) d -> p ko d", p=P)
    wot_v = wot_d[:].rearrange("(h p) o -> p h o", p=P)
    bq_v = bq_d[:].rearrange("(h p) -> p h", p=P)
    bk_v = bk_d[:].rearrange("(h p) -> p h", p=P)
    mask_v = mask_d[:].rearrange("b (ko p) -> p b ko", p=P)

    with tile.TileContext(nc) as tc:
        with (
            tc.tile_pool(name="const", bufs=1) as const,
            tc.tile_pool(name="big", bufs=1) as big,
            tc.tile_pool(name="xch", bufs=xch_bufs) as xch,
            tc.tile_pool(name="ptp", bufs=pt_bufs) as ptp,
            tc.tile_pool(name="accp", bufs=acc_bufs) as accp,
            tc.tile_pool(name="nrm", bufs=2) as nrm,
            tc.tile_pool(name="ob", bufs=obp_bufs) as obp,
            tc.tile_pool(name="ps", bufs=ps_bufs, space="PSUM") as psp,
            tc.tile_pool(name="pss", bufs=opts.get("pss_bufs", 2),
                         space="PSUM") as pssp,
            tc.tile_pool(name="dscr", bufs=4, space="DRAM") as dscr,
        ):
            Ident = mybir.ActivationFunctionType.Identity
            Exp = mybir.ActivationFunctionType.Exp

            def ps_tile():
                return psp.tile([P, 512], fp32, tag="ps", name="ps")

            # ---- resident constants -------------------------------------
            wq_sb = const.tile([P, KO, DLOC], bf16)
            wk_sb = const.tile([P, KO, DLOC], bf16)
            wv_sb = const.tile([P, KO, DLOC], bf16)
            wo_sb = const.tile([P, H_LOC, HIDDEN], bf16)
            bq_sb = const.tile([P, H_LOC], fp32)
            bk_sb = const.tile([P, H_LOC], fp32)
            mask_sb = const.tile([P, B, NKT], fp32)
            ones_sb = const.tile([P, 1], bf16)
            ones_row = const.tile([1, P], bf16)  # lhsT for rcp broadcast MM

            # Startup: xc0 streams on the ACT-triggered queue while weights
            # stream on the SP queue, in 2-ko pieces interleaved so the first
            # Q matmuls can begin after ~one piece of each.
            xc0 = xch.tile([P, KO, CHW], bf16, tag="xc", name="xc")
            for ko in range(0, KO, 2):
                nc.sync.dma_start(
                    wq_sb[:, ko:ko + 2, :], wqt_v[:, ko:ko + 2, :]
                )
                nc.scalar.dma_start(
                    xc0[:, ko:ko + 2, :], xt_v[:, ko:ko + 2, 0:CHW]
                )
            for ko in range(0, KO, 8):
                nc.sync.dma_start(wk_sb[:, ko:ko + 8, :], wkt_v[:, ko:ko + 8, :])
            for ko in range(0, KO, 8):
                nc.sync.dma_start(wv_sb[:, ko:ko + 8, :], wvt_v[:, ko:ko + 8, :])
            nc.sync.dma_start(bq_sb[:], bq_v)
            nc.sync.dma_start(bk_sb[:], bk_v)
            nc.sync.dma_start(mask_sb[:], mask_v)
            # wo is first needed ~90us in (output projections); its load is
            # deferred into emit_body so it doesn't crowd startup bandwidth
            nc.vector.memset(ones_sb[:], 1.0)
            nc.vector.memset(ones_row[:], 1.0)

            # ---- big activation buffers ---------------------------------
            qt_sb = big.tile([P, H_LOC, T], bf16)     # Q.T (d on partitions)
            kt_sb = big.tile([P, H_LOC, T], bf16)     # K.T
            v_sb = big.tile([P, T // P, DLOC], bf16)  # V natural
            ctx_sb = big.tile([P, H_LOC, T], bf16)    # normalized ctx.T

            # ---- filler queues ------------------------------------------
            # Each queue entry emits exactly one PE matmul (plus trailing
            # drains / DMAs that complete its accumulation group). Index
            # pointers (not pops) so emission guards can reference positions.
            chunkq = []
            cqi = [0]
            outq = []
            oqi = [0]
            out_reserve = opts.get("out_reserve", 2)

            def pull_filler(n):
                # keep a few freshest outproj steps unpulled so PE never
                # reaches an outproj matmul before its ctx norm lands
                for _ in range(n):
                    if cqi[0] < len(chunkq):
                        chunkq[cqi[0]]()
                        cqi[0] += 1
                    elif len(outq) - oqi[0] > out_reserve:
                        outq[oqi[0]]()
                        oqi[0] += 1
                    else:
                        break

            def drain_cq_to(pos):
                while cqi[0] < pos:
                    chunkq[cqi[0]]()
                    cqi[0] += 1

            def drain_oq():
                while oqi[0] < len(outq):
                    outq[oqi[0]]()
                    oqi[0] += 1

            # ---- phase 1: Q/K/V projections for one 512-token chunk -----
            # Chunks 0-3 emit Q+K+V together ("all"); chunks 4-7 are split
            # into a K+V pass and a separate Q pass (own xc load) so the Q
            # work for chunks 6-7 can defer into the early b1 attention
            # blocks as cheap filler (almost no drain traffic).
            xc_tiles = {("a", 0): xc0}
            _next_key = {}

            def load_chunk(key):
                kind, c = key
                if key in xc_tiles or c >= CH:
                    return
                if abl_no_xt:
                    xc_tiles[key] = xc0
                    return
                cs = slice(c * CHW, (c + 1) * CHW)
                xc = xch.tile([P, KO, CHW], bf16, tag="xc", name="xc")
                nc.sync.dma_start(xc[:, :8, :], xt_v[:, :8, cs])
                nc.sync.dma_start(xc[:, 8:, :], xt_v[:, 8:, cs])
                xc_tiles[key] = xc

            def chunk_steps(c, mode="all"):
                st = {}
                cs = slice(c * CHW, (c + 1) * CHW)
                drain_act = c < 4  # ACT is idle before attention starts
                steps = []
                key = ("a" if mode == "all" else mode, c)

                def first_pre():
                    load_chunk(key)
                    st["xc"] = xc_tiles[key]
                    nxt = _next_key.get(key)
                    if nxt is not None:
                        load_chunk(nxt)  # prefetch one pass ahead

                def qk_group(w_sb, b_sb, dst, h, pre=None):
                    g = {}
                    hd = slice(h * P, (h + 1) * P)

                    def mk(ko):
                        def run():
                            if ko == 0:
                                if pre is not None:
                                    pre()
                                g["ps"] = ps_tile()
                            nc.tensor.matmul(
                                g["ps"][:], w_sb[:, ko, hd], st["xc"][:, ko, :],
                                start=(ko == 0), stop=(ko == KO - 1),
                            )
                            if ko == KO - 1:
                                if drain_act:
                                    nc.scalar.activation(
                                        dst[:, h, cs], g["ps"][:], Ident,
                                        bias=b_sb[:, h:h + 1],
                                    )
                                else:
                                    nc.vector.tensor_scalar_add(
                                        dst[:, h, cs], g["ps"][:], b_sb[:, h:h + 1]
                                    )
                        return run

                    return [mk(ko) for ko in range(KO)]

                def v_group(tt):
                    g = {}

                    def mk(ko):
                        def run():
                            if ko == 0:
                                g["ps"] = ps_tile()
                            nc.tensor.matmul(
                                g["ps"][:, :DLOC],
                                st["xc"][:, ko, tt * P:(tt + 1) * P],
                                wv_sb[:, ko, :],
                                start=(ko == 0), stop=(ko == KO - 1),
                            )
                            if ko == KO - 1:
                                nc.vector.tensor_copy(
                                    v_sb[:, c * (CHW // P) + tt, :],
                                    g["ps"][:, :DLOC],
                                )
                        return run

                    return [mk(ko) for ko in range(KO)]

                if mode == "all" and c == 0:
                    # Interleave the two heads at ko granularity so PE
                    # consumes each freshly-DMA'd 2-ko piece with 4 matmuls
                    # instead of 2 — keeps pace with the startup stream.
                    q0 = qk_group(wq_sb, bq_sb, qt_sb, 0, pre=first_pre)
                    q1 = qk_group(wq_sb, bq_sb, qt_sb, 1)
                    for a, bstep in zip(q0, q1):
                        steps += [a, bstep]
                    k0 = qk_group(wk_sb, bk_sb, kt_sb, 0)
                    k1 = qk_group(wk_sb, bk_sb, kt_sb, 1)
                    for a, bstep in zip(k0, k1):
                        steps += [a, bstep]
                elif mode == "all":
                    steps += qk_group(wq_sb, bq_sb, qt_sb, 0, pre=first_pre)
                    steps += qk_group(wq_sb, bq_sb, qt_sb, 1)
                    steps += qk_group(wk_sb, bk_sb, kt_sb, 0)
                    steps += qk_group(wk_sb, bk_sb, kt_sb, 1)
                elif mode == "kv":
                    steps += qk_group(wk_sb, bk_sb, kt_sb, 0, pre=first_pre)
                    steps += qk_group(wk_sb, bk_sb, kt_sb, 1)
                elif mode == "q":
                    steps += qk_group(wq_sb, bq_sb, qt_sb, 0, pre=first_pre)
                    steps += qk_group(wq_sb, bq_sb, qt_sb, 1)
                    return steps
                for tt in range(CHW // P):
                    steps += v_group(tt)
                return steps

            # ---- output projection for one 128-token tile ---------------
            # dr: which engine drains the psum -> bf16 output buffer.
            #   "act": always ACT (used for b0 tiles drained while ACT is
            #          otherwise idle between the attention phases)
            #   "alt": mostly DVE with every 4th on ACT (b1 tiles drained
            #          while ACT is busy with exps)
            def outproj_steps(tt, dr):
                ts_ = slice(tt * P, (tt + 1) * P)
                steps = []
                for oi in range(HIDDEN // 512):
                    g = {}
                    os_ = slice(oi * 512, (oi + 1) * 512)
                    if dr == "act":
                        cp_eng = "act"
                    elif dr == "alt":
                        cp_eng = (
                            "pool" if b1_pool
                            else ("act" if oi % alt_mod == alt_mod - 1 else "dve")
                        )
                    else:  # tail
                        cp_eng = "act" if oi % 2 else "dve"
                    st_eng = nc.scalar if (dr == "tail" and oi % 2 == 0) else nc.sync

                    def mk(h, oi=oi, os_=os_, g=g, cp_eng=cp_eng, st_eng=st_eng,
                           dr=dr):
                        def run():
                            if h == 0:
                                g["ps"] = ps_tile()
                            nc.tensor.matmul(
                                g["ps"][:], ctx_sb[:, h, ts_], wo_sb[:, h, os_],
                                start=(h == 0), stop=(h == H_LOC - 1),
                            )
                            if h == H_LOC - 1:
                                ob = obp.tile(
                                    [P, 512], out_dt, tag="ob", name="ob"
                                )
                                if dr == "tail" and tail_split:
                                    # half-width drains on both engines and
                                    # stores on both queues: the last stores
                                    # start ~a half-copy earlier
                                    ha = slice(os_.start, os_.start + 256)
                                    hb = slice(os_.start + 256, os_.stop)
                                    nc.vector.tensor_copy(
                                        ob[:, 0:256], g["ps"][:, 0:256]
                                    )
                                    if not abl_no_store:
                                        nc.sync.dma_start(
                                            out_d[ts_, ha], ob[:, 0:256]
                                        )
                                    nc.scalar.copy(
                                        ob[:, 256:512], g["ps"][:, 256:512]
                                    )
                                    if not abl_no_store:
                                        nc.scalar.dma_start(
                                            out_d[ts_, hb], ob[:, 256:512]
                                        )
                                    return
                                if cp_eng == "act":
                                    nc.scalar.copy(ob[:], g["ps"][:])
                                elif cp_eng == "pool":
                                    nc.gpsimd.tensor_copy(ob[:], g["ps"][:])
                                else:
                                    nc.vector.tensor_copy(ob[:], g["ps"][:])
                                if not abl_no_store:
                                    st_eng.dma_start(out_d[ts_, os_], ob[:])
                        return run

                    for h in range(H_LOC):
                        steps.append(mk(h))
                return steps

            # ---- attention for one (batch, head, q-tile) ----------------
            # The normalization tail of block k is deferred into block k+1
            # (s1 at kt==2, s2 at kt==6) so the DVE reciprocal chain and the
            # PE broadcast matmul never make either engine wait: by the time
            # each step runs, its producer on the other engine is long done.
            def attention2(b, h, qi, fill, prev, stg=None):
                stg = stagger if stg is None else stg
                hd = slice(h * P, (h + 1) * P)
                qs = slice(b * S + qi * 512, b * S + (qi + 1) * 512)
                st = {}
                pts = [None] * NKT
                acc = [None, None]

                AW = 1024 if pair_exp else 512

                def st_exp(kt):
                    ks = slice(b * S + kt * P, b * S + (kt + 1) * P)
                    if pair_exp:
                        if kt % 2 == 1:
                            return  # emitted with the even half
                        ks2 = slice(b * S + (kt + 1) * P, b * S + (kt + 2) * P)
                        ps_d = psp.tile(
                            [P, 1024], fp32, tag="psd", name="psd", bufs=2
                        )
                        nc.tensor.matmul(
                            ps_d[:, 0:512], kt_sb[:, h, ks], qt_sb[:, h, qs],
                            start=True, stop=True,
                        )
                        nc.tensor.matmul(
                            ps_d[:, 512:1024], kt_sb[:, h, ks2], qt_sb[:, h, qs],
                            start=True, stop=True,
                        )
                        if abl_half_exp and kt % 4 == 2:
                            # timing ablation: PE work identical, exp skipped
                            pts[kt] = pts[kt - 2]
                            pts[kt + 1] = pts[kt - 1]
                            return
                        pt = ptp.tile([P, 1024], bf16, tag="pt", name="pt")
                        nc.scalar.activation(pt[:], ps_d[:], Exp, scale=SCALE)
                        pts[kt] = (pt, 0)
                        pts[kt + 1] = (pt, 512)
                        return
                    ps_s = ps_tile()
                    nc.tensor.matmul(
                        ps_s[:], kt_sb[:, h, ks], qt_sb[:, h, qs],
                        start=True, stop=True,
                    )
                    pt = ptp.tile([P, 512], bf16, tag="pt", name="pt")
                    nc.scalar.activation(
                        pt[:], ps_s[:], Exp,
                        bias=mask_sb[:, b, kt:kt + 1], scale=SCALE,
                    )
                    pts[kt] = (pt, 0)

                def consume(kt):
                    if kt == 0:
                        st["ctx"] = ps_tile()
                        if sums_pe:
                            st["sum"] = pssp.tile(
                                [P, 512], fp32, tag="pssum", name="pssum"
                            )
                    pt, off = pts[kt]
                    nc.tensor.matmul(
                        st["ctx"][:], v_sb[:, b * NKT + kt, hd],
                        pt[:, off:off + 512],
                        start=(kt == 0), stop=(kt == NKT - 1),
                    )
                    if sums_pe:
                        nc.tensor.matmul(
                            st["sum"][0:1, :], ones_sb[:], pt[:, off:off + 512],
                            start=(kt == 0), stop=(kt == NKT - 1),
                        )
                        return
                    if abl_no_acc or sums_tree2:
                        return
                    if pair_exp and kt % 2 == 0:
                        return  # accumulate the pair once, at the odd half
                    i = (kt // (2 if pair_exp else 1)) % 2
                    aeng = nc.gpsimd if acc_pool else nc.vector
                    if acc[i] is None:
                        acc[i] = accp.tile(
                            [P, AW], bf16, tag=f"acc{i}", name=f"acc{i}"
                        )
                        aeng.tensor_copy(acc[i][:], pt[:])
                    else:
                        aeng.tensor_add(acc[i][:], acc[i][:], pt[:])

                if attn_batch4:
                    assert pair_exp and stg % 4 == 0
                    for g in range(0, NKT, 4):
                        st_exp(g)
                        st_exp(g + 2)
                        pull_filler(fill * 4)
                        for sn, sat in (("s0", s0_at), ("s1", s1_at),
                                        ("s2", s2_at)):
                            if g <= sat < g + 4 and prev.get(sn):
                                prev[sn]()
                        if g >= stg:
                            for kt in range(g - stg, g - stg + 4):
                                consume(kt)
                    for kt in range(NKT - stg, NKT):
                        consume(kt)
                else:
                    for kt in range(NKT):
                        st_exp(kt)
                        pull_filler(fill)
                        if kt == s0_at and prev.get("s0"):
                            prev["s0"]()
                        if kt == s1_at and prev.get("s1"):
                            prev["s1"]()
                        if kt == s2_at and prev.get("s2"):
                            prev["s2"]()
                        if kt >= stg:
                            consume(kt - stg)
                    for kt in range(NKT - stg, NKT):
                        consume(kt)

                if abl_no_acc:
                    def s2_raw():
                        nc.vector.tensor_copy(ctx_sb[:, h, qs], st["ctx"][:])
                        if h == H_LOC - 1 and not abl_no_outproj:
                            dr = "act" if b == 0 else (
                                "tail" if qi == NQ - 1 else "alt")
                            for tt in range(b * 16 + qi * 4,
                                            b * 16 + qi * 4 + 4):
                                outq.extend(outproj_steps(tt, dr))
                    return {"s1": None, "s2": s2_raw}
                if sums_tree2:
                    ctxu = nrm.tile([P, 512], fp32, tag="ctxu", name="ctxu")
                    nc.vector.tensor_copy(ctxu[:], st["ctx"][:])
                    cur = {}
                    pair_tiles = [pts[2 * i][0] for i in range(NKT // 2)]

                    def s0():
                        lvl = pair_tiles
                        li = 0
                        while len(lvl) > 1:
                            nxt = []
                            for i in range(0, len(lvl), 2):
                                t = accp.tile(
                                    [P, AW], bf16, tag=f"tl{li}",
                                    name=f"tl{li}",
                                    bufs=(opts.get("tl0_bufs", 4)
                                          if li == 0 else 3),
                                )
                                nc.vector.tensor_add(t[:], lvl[i][:], lvl[i + 1][:])
                                nxt.append(t)
                            lvl = nxt
                            li += 1
                        accf2 = accp.tile([P, 512], bf16, tag="accs",
                                          name="accs", bufs=2)
                        nc.vector.tensor_add(
                            accf2[:], lvl[0][:, 0:512], lvl[0][:, 512:1024]
                        )
                        cur["accf"] = accf2

                    def s1t():
                        ps_sum = ps_tile()
                        nc.tensor.matmul(
                            ps_sum[0:1, :], ones_sb[:], cur["accf"][:],
                            start=True, stop=True,
                        )
                        rcp = nrm.tile([1, 512], fp32, tag="rcp", name="rcp")
                        nc.vector.reciprocal(rcp[:], ps_sum[0:1, :])
                        if norm_dma:
                            rdr = dscr.tile([1, 512], fp32, tag="rdr",
                                            name="rdr")
                            rbc = nrm.tile([P, 512], fp32, tag="rbc",
                                           name="rbc")
                            nc.sync.dma_start(rdr[:], rcp[:])
                            nc.sync.dma_start(
                                rbc[:], rdr[:].to_broadcast((P, 512))
                            )
                            cur["rbc"] = rbc
                            return
                        rcpb = nrm.tile([1, 512], bf16, tag="rcpb", name="rcpb")
                        nc.vector.tensor_copy(rcpb[:], rcp[:])
                        cur["rcpb"] = rcpb

                    def s2t():
                        if norm_dma:
                            nc.vector.tensor_mul(
                                ctx_sb[:, h, qs], ctxu[:], cur["rbc"][:]
                            )
                        else:
                            ps_rbc = ps_tile()
                            nc.tensor.matmul(
                                ps_rbc[:], ones_row[:], cur["rcpb"][:],
                                start=True, stop=True,
                            )
                            nc.vector.tensor_mul(
                                ctx_sb[:, h, qs], ctxu[:], ps_rbc[:]
                            )
                        if h == H_LOC - 1 and not abl_no_outproj:
                            if b == 0:
                                dr = "act"
                            elif qi == NQ - 1:
                                dr = "tail"
                            else:
                                dr = "alt"
                            for tt in range(b * 16 + qi * 4,
                                            b * 16 + qi * 4 + 4):
                                outq.extend(outproj_steps(tt, dr))

                    return {"s0": s0, "s1": s1t, "s2": s2t}
                if sums_pe:
                    accf = None
                else:
                    aeng2 = nc.gpsimd if acc_pool else nc.vector
                    aeng2.tensor_add(acc[0][:], acc[0][:], acc[1][:])
                    if pair_exp:
                        accs = accp.tile([P, 512], bf16, tag="accs", name="accs")
                        nc.vector.tensor_add(
                            accs[:], acc[0][:, 0:512], acc[0][:, 512:1024]
                        )
                        accf = accs
                    else:
                        accf = acc[0]
                ctxu = nrm.tile([P, 512], fp32, tag="ctxu", name="ctxu")
                nc.vector.tensor_copy(ctxu[:], st["ctx"][:])
                cur = {}

                def s1():
                    if sums_pe:
                        rcp = nrm.tile([1, 512], fp32, tag="rcp", name="rcp")
                        nc.vector.reciprocal(rcp[:], st["sum"][0:1, :])
                        rcpb = nrm.tile([1, 512], bf16, tag="rcpb", name="rcpb")
                        nc.vector.tensor_copy(rcpb[:], rcp[:])
                        cur["rcpb"] = rcpb
                        return
                    if norm_pool:
                        import concourse.bass_isa as bass_isa
                        rs = nrm.tile([P, 512], fp32, tag="rs", name="rs")
                        nc.gpsimd.partition_all_reduce(
                            rs[:], accf[:], P, bass_isa.ReduceOp.add
                        )
                        rcp = nrm.tile([1, 512], fp32, tag="rcp", name="rcp")
                        nc.vector.reciprocal(rcp[:], rs[0:1, :])
                        rb = nrm.tile([P, 512], fp32, tag="rb2", name="rb2")
                        nc.gpsimd.partition_broadcast(rb[:], rcp[:], P)
                        cur["rb"] = rb
                        return
                    ps_sum = ps_tile()
                    nc.tensor.matmul(
                        ps_sum[0:1, :], ones_sb[:], accf[:],
                        start=True, stop=True,
                    )
                    rcp = nrm.tile([1, 512], fp32, tag="rcp", name="rcp")
                    nc.vector.reciprocal(rcp[:], ps_sum[0:1, :])
                    if norm_dma:
                        # bounce through DRAM to broadcast 1/sums across
                        # partitions; latency hidden by the s1->s2 deferral
                        rdr = dscr.tile([1, 512], fp32, tag="rdr", name="rdr")
                        rbc = nrm.tile([P, 512], fp32, tag="rbc", name="rbc")
                        nc.sync.dma_start(rdr[:], rcp[:])
                        nc.sync.dma_start(rbc[:], rdr[:].to_broadcast((P, 512)))
                        cur["rbc"] = rbc
                    else:
                        rcpb = nrm.tile([1, 512], bf16, tag="rcpb", name="rcpb")
                        nc.vector.tensor_copy(rcpb[:], rcp[:])
                        cur["rcpb"] = rcpb

                def s2():
                    if abl_raw_ctx:
                        pass  # ctxu already holds raw ctx; no norm multiply
                    elif norm_pool:
                        nc.vector.tensor_mul(
                            ctx_sb[:, h, qs], ctxu[:], cur["rb"][:]
                        )
                    elif norm_dma:
                        nc.vector.tensor_mul(
                            ctx_sb[:, h, qs], ctxu[:], cur["rbc"][:]
                        )
                    else:
                        ps_rbc = ps_tile()
                        nc.tensor.matmul(
                            ps_rbc[:], ones_row[:], cur["rcpb"][:],
                            start=True, stop=True,
                        )
                        nc.vector.tensor_mul(ctx_sb[:, h, qs], ctxu[:], ps_rbc[:])
                    if h == H_LOC - 1 and not abl_no_outproj:
                        if b == 0:
                            dr = "act"
                        elif qi == NQ - 1:
                            dr = "tail"
                        else:
                            dr = "alt"
                        for tt in range(b * 16 + qi * 4, b * 16 + qi * 4 + 4):
                            outq.extend(outproj_steps(tt, dr))

                return {"s1": s1, "s2": s2}

            # ---- streamed attention (global exp->consume lag) ------------
            def attention_stream(blks, fill, prev, pre_hooks=None):
                assert pair_exp and sums_tree2 and s0_at >= stagger
                sts = [{"pts": [None] * NKT} for _ in blks]
                stages = {-1: prev}

                def st_exp(bi, kt):
                    if kt % 2 == 1:
                        return
                    b, h, qi = blks[bi]
                    qs = slice(b * S + qi * 512, b * S + (qi + 1) * 512)
                    st = sts[bi]
                    ks = slice(b * S + kt * P, b * S + (kt + 1) * P)
                    ks2 = slice(b * S + (kt + 1) * P, b * S + (kt + 2) * P)
                    ps_d = psp.tile([P, 1024], fp32, tag="psd", name="psd",
                                    bufs=2)
                    nc.tensor.matmul(
                        ps_d[:, 0:512], kt_sb[:, h, ks], qt_sb[:, h, qs],
                        start=True, stop=True,
                    )
                    nc.tensor.matmul(
                        ps_d[:, 512:1024], kt_sb[:, h, ks2], qt_sb[:, h, qs],
                        start=True, stop=True,
                    )
                    pt = ptp.tile([P, 1024], bf16, tag="pt", name="pt")
                    nc.scalar.activation(pt[:], ps_d[:], Exp, scale=SCALE)
                    st["pts"][kt] = (pt, 0)
                    st["pts"][kt + 1] = (pt, 512)

                def finalize(bi):
                    b, h, qi = blks[bi]
                    qs = slice(b * S + qi * 512, b * S + (qi + 1) * 512)
                    st = sts[bi]
                    ctxu = nrm.tile([P, 512], fp32, tag="ctxu", name="ctxu")
                    nc.vector.tensor_copy(ctxu[:], st["ctx"][:])
                    cur = {}
                    pair_tiles = [st["pts"][2 * i][0] for i in range(NKT // 2)]

                    def s0():
                        lvl = pair_tiles
                        li = 0
                        while len(lvl) > 1:
                            nxt = []
                            for i in range(0, len(lvl), 2):
                                t = accp.tile(
                                    [P, 1024], bf16, tag=f"tl{li}",
                                    name=f"tl{li}",
                                    bufs=(opts.get("tl0_bufs", 4)
                                          if li == 0 else 3),
                                )
                                nc.vector.tensor_add(
                                    t[:], lvl[i][:], lvl[i + 1][:]
                                )
                                nxt.append(t)
                            lvl = nxt
                            li += 1
                        accf2 = accp.tile([P, 512], bf16, tag="accs",
                                          name="accs", bufs=2)
                        nc.vector.tensor_add(
                            accf2[:], lvl[0][:, 0:512], lvl[0][:, 512:1024]
                        )
                        cur["accf"] = accf2

                    def s1():
                        ps_sum = ps_tile()
                        nc.tensor.matmul(
                            ps_sum[0:1, :], ones_sb[:], cur["accf"][:],
                            start=True, stop=True,
                        )
                        rcp = nrm.tile([1, 512], fp32, tag="rcp", name="rcp")
                        nc.vector.reciprocal(rcp[:], ps_sum[0:1, :])
                        rcpb = nrm.tile([1, 512], bf16, tag="rcpb",
                                        name="rcpb")
                        nc.vector.tensor_copy(rcpb[:], rcp[:])
                        cur["rcpb"] = rcpb

                    def s2():
                        ps_rbc = ps_tile()
                        nc.tensor.matmul(
                            ps_rbc[:], ones_row[:], cur["rcpb"][:],
                            start=True, stop=True,
                        )
                        nc.vector.tensor_mul(
                            ctx_sb[:, h, qs], ctxu[:], ps_rbc[:]
                        )
                        if h == H_LOC - 1 and not abl_no_outproj:
                            if b == 0:
                                dr = "act"
                            elif qi == NQ - 1:
                                dr = "tail"
                            else:
                                dr = "alt"
                            for tt in range(b * 16 + qi * 4,
                                            b * 16 + qi * 4 + 4):
                                outq.extend(outproj_steps(tt, dr))

                    stages[bi] = {"s0": s0, "s1": s1, "s2": s2}

                def consume(bi, kt):
                    b, h, qi = blks[bi]
                    hd = slice(h * P, (h + 1) * P)
                    st = sts[bi]
                    if kt == 0:
                        st["ctx"] = ps_tile()
                    pt, off = st["pts"][kt]
                    nc.tensor.matmul(
                        st["ctx"][:], v_sb[:, b * NKT + kt, hd],
                        pt[:, off:off + 512],
                        start=(kt == 0), stop=(kt == NKT - 1),
                    )
                    if kt == NKT - 1:
                        finalize(bi)

                seq = [(bi, kt) for bi in range(len(blks))
                       for kt in range(NKT)]
                for i, (bi, kt) in enumerate(seq):
                    if kt == 0 and pre_hooks and bi in pre_hooks:
                        pre_hooks[bi]()
                    st_exp(bi, kt)
                    pull_filler(fill)
                    pv = stages.get(bi - 1)
                    if pv:
                        if kt == s0_at and pv.get("s0"):
                            pv["s0"]()
                        if kt == s1_at and pv.get("s1"):
                            pv["s1"]()
                        if kt == s2_at and pv.get("s2"):
                            pv["s2"]()
                    j = i - stagger
                    if j >= 0:
                        consume(*seq[j])
                for j in range(len(seq) - stagger, len(seq)):
                    consume(*seq[j])
                return stages[len(blks) - 1]

            def finish(prev, spacing):
                if prev.get("s0"):
                    prev["s0"]()
                pull_filler(spacing)
                if prev.get("s1"):
                    prev["s1"]()
                pull_filler(spacing)
                if prev.get("s2"):
                    prev["s2"]()

            # ---- global schedule ----------------------------------------
            def emit_body():
                if abl_proj_only:
                    for c in range(4):
                        for s in chunk_steps(c, "all"):
                            s()
                    for c in range(4, CH):
                        for s in chunk_steps(c, "kv"):
                            s()
                    for c in range(4, CH):
                        for s in chunk_steps(c, "q"):
                            s()
                    return
                if abl_attn_only:
                    # fake producers for Q/K/V so attention has dependencies
                    nc.vector.memset(qt_sb[:], 0.001)
                    nc.vector.memset(kt_sb[:], 0.001)
                    nc.vector.memset(v_sb[:], 0.001)
                    nc.sync.dma_start(wo_sb[:], wot_v)
                    prev = {}
                    for b in range(B):
                        for qi in range(NQ):
                            for h in range(H_LOC):
                                prev = attention2(b, h, qi, 0, prev)
                    finish(prev, 2)
                    drain_oq()
                    return
                keys = [("a", c) for c in range(4)]
                if defer_q:
                    keys += [("kv", c) for c in range(4, CH)]
                    keys += [("q", c) for c in range(4, CH)]
                else:
                    keys += [("a", c) for c in range(4, CH)]
                for ka, kb in zip(keys, keys[1:]):
                    _next_key[ka] = kb
                for c in range(4):
                    for s in chunk_steps(c, "all"):
                        s()
                qpos = {}
                if defer_q:
                    for c in range(4, CH):
                        chunkq.extend(chunk_steps(c, "kv"))
                    for c in range(4, CH):
                        chunkq.extend(chunk_steps(c, "q"))
                        qpos[c] = len(chunkq)
                else:
                    for c in range(4, CH):
                        chunkq.extend(chunk_steps(c, "all"))
                    for c in range(4, CH):
                        qpos[c] = len(chunkq) if c >= 5 else 0
                    qpos[5] = len(chunkq)
                nc.sync.dma_start(wo_sb[:], wot_v)
                prev = {}
                if stream_attn:
                    b0_blks = [(0, h, qi) for qi in range(NQ)
                               for h in range(H_LOC)]
                    prev = attention_stream(b0_blks, fill_b0, prev)
                else:
                    for qi in range(NQ):
                        for h in range(H_LOC):
                            prev = attention2(0, h, qi, fill_b0, prev)
                finish(prev, 16)
                # drain phase: emit K/V for chunks 4-7 plus Q for chunks 4-5,
                # interleaving the b0 output projections (their psum drains
                # ride the idle ACT engine); Q for chunks 6-7 stays queued as
                # cheap filler for the early b1 blocks
                dstop = qpos[drain_to]
                while cqi[0] < dstop:
                    for _ in range(drain_ratio):
                        if cqi[0] < dstop:
                            chunkq[cqi[0]]()
                            cqi[0] += 1
                    if oqi[0] < len(outq):
                        outq[oqi[0]]()
                        oqi[0] += 1
                prev = {}
                if stream_attn:
                    b1_blks = [(1, h, qi) for qi in range(NQ)
                               for h in range(H_LOC)]
                    hooks = {
                        qi * H_LOC: (lambda q=qi: drain_cq_to(qpos[4 + q]))
                        for qi in range(NQ)
                    }
                    prev = attention_stream(b1_blks, fill_b1, prev,
                                            pre_hooks=hooks)
                else:
                    for qi in range(NQ):
                        drain_cq_to(qpos[4 + qi])  # this qi's Q must be emitted
                        for h in range(H_LOC):
                            last = qi == NQ - 1 and h == H_LOC - 1
                            prev = attention2(
                                1, h, qi, fill_b1, prev,
                                stg=(4 if attn_batch4 else 2) if last else None,
                            )
                finish(prev, 2)
                drain_cq_to(len(chunkq))
                drain_oq()

            if loop_k is None:
                emit_body()
            else:
                with tc.For_i(0, loop_k, 1):
                    emit_body()

    _split_multi_waits(nc)
    return nc


# Final tuned configuration (HW-validated via slope benchmarking + cost model):
#  - out_bf16: bf16 partial outputs (halves store traffic; host sums in fp64)
#  - obp_bufs=8: deep store pipeline (phase-3 tail was store-bound)
#  - split_in: split startup DMAs so the first matmuls start early
#  - norm2: drain ctx psum early; reciprocal broadcast off the critical path
#  - act_lite: ACT engine reserved for exps; bias-adds/drains on DVE
#  - sched2 + stagger: interleave projection/attention/out-projection emission
#    and software-pipeline the attention loop so exp latency never stalls PE
_DEFAULT_OPTS = dict(
    out_bf16=True, obp_bufs=8, split_in=True, norm2=True,
    act_lite=True, sched2=True, stagger=2, early_x=True,
)

# v2 schedule (see _build_nc2): DVE-accumulated softmax sums + fine-grained
# filler interleave. Defaults here are the sim-tuned configuration.
_DEFAULT_OPTS2 = dict()
# With an all-zero attention mask the per-key exp bias is unused, so each
# pair of score tiles can share one [128,1024] exp (HW A/B: ~3% faster).
# alt_mod=2: with paired exps ACT has slack, so b1 outproj drains split
# evenly between DVE and ACT (HW A/B: slightly faster than 3-of-4 on DVE).
# v3 addition (HW paired A/B, 2026-08-11): sums_tree2 — softmax
# denominators via a DVE adder tree over the block's retained exp tiles,
# deferred into the next block (s0), replacing the per-exp accumulate
# chain. The chain serialized DVE behind each fresh exp (ACT->DVE sem
# coupling + queue head-of-line blocking of the outproj drains); the tree
# runs on long-ready inputs. Paired A/B: -17us vs the v2 chain.
# (attn_batch4 / norm_dma / sums_pe / stream_attn were also built and
# HW-benched; none beat plain sums_tree2 — see bench logs + memory notes.)
_PAIR_OPTS = dict(
    pair_exp=True, pt_bufs=11, xch_bufs=3, tl0_bufs=4, obp_bufs=8,
    ps_bufs=4, alt_mod=2, drain_to=4, drain_ratio=2,
    sums_tree2=True, s0_at=1, s1_at=6, s2_at=12,
    # fill_b0=2 / fill_b1=3: pull more filler matmuls per kt inside the
    # attention blocks (paired A/B: -3us and -27us). fill_b1=3 drains the
    # b1 output-projection queue during the blocks instead of the serial
    # end tail, which the timeline sim showed as the largest remaining
    # PE-idle stretch.
    fill_b0=2, fill_b1=3,
)


def _get_nc(zero_mask=False):
    key = ("nc", zero_mask)
    if key not in _CACHE:
        if os.environ.get("MHA_KERNEL_V", "2") == "1":
            _CACHE[key] = _build_nc(**_DEFAULT_OPTS)
        else:
            opts = dict(_DEFAULT_OPTS2)
            if zero_mask:
                opts.update(_PAIR_OPTS)
            _CACHE[key] = _build_nc2(**opts)
    return _CACHE[key]


def kernel(**inputs):
    hs = np.asarray(inputs["hidden_states"], dtype=np.float32)
    mask = np.asarray(inputs["attention_mask"], dtype=np.float32)
    Wq = np.asarray(inputs["Wq"], dtype=np.float32)
    bq = np.asarray(inputs["bq"], dtype=np.float32)
    Wk = np.asarray(inputs["Wk"], dtype=np.float32)
    bk = np.asarray(inputs["bk"], dtype=np.float32)
    Wv = np.asarray(inputs["Wv"], dtype=np.float32)
    bv = np.asarray(inputs["bv"], dtype=np.float32)
    Wo = np.asarray(inputs["Wo"], dtype=np.float32)
    bo = np.asarray(inputs["bo"], dtype=np.float32)

    x = hs.reshape(T, HIDDEN)
    xt = np.ascontiguousarray(x.T).astype(BF16NP)
    mask2 = np.ascontiguousarray(mask.reshape(B, S))

    in_maps = []
    for c in range(N_CORES):
        rs = slice(c * DLOC, (c + 1) * DLOC)
        in_maps.append({
            "xt": xt,
            "wqt": np.ascontiguousarray(Wq[rs, :].T).astype(BF16NP),
            "wkt": np.ascontiguousarray(Wk[rs, :].T).astype(BF16NP),
            "wvt": np.ascontiguousarray(Wv[rs, :].T).astype(BF16NP),
            "wot": np.ascontiguousarray(Wo[:, rs].T).astype(BF16NP),
            "bq": np.ascontiguousarray(bq[rs]),
            "bk": np.ascontiguousarray(bk[rs]),
            "mask": mask2,
        })

    from concourse.bass_utils import run_bass_kernel_spmd

    nc = _get_nc(zero_mask=not np.any(mask2))
    trace = bool(int(os.environ.get("MHA_KERNEL_TRACE", "0")))

    def _run():
        return run_bass_kernel_spmd(
            nc, in_maps, core_ids=list(range(N_CORES)), trace=trace,
            **({"trace_cores": list(range(N_CORES))} if trace else {}),
        )

    try:
        res = _run()
    except Exception:
        # transient device errors (e.g. NRT_EXEC_UNIT_UNRECOVERABLE after a
        # prior process wedged the core) have been observed to clear on a
        # retry; one retry costs nothing on the success path
        res = _run()
    _CACHE["last_results"] = res

    out = np.sum(
        np.stack([r["out"] for r in res.results]), axis=0, dtype=np.float64
    )
    out += bv.astype(np.float64) @ Wo.T.astype(np.float64) + bo
    return out.astype(np.float32).reshape(B, S, HIDDEN)

